# revision 1
# baseline (speedup 1.0000x reference)
"""BEV deformable cross-attention kernel for 8 Trainium2 NeuronCores.

Strategy (per core): data-parallel over (B x K-half): core c handles batch
b = c//2 and modes k in {3*(c%2) .. +3}, i.e. 36 queries, 288 sample points.

Key algebraic move: grid_sample(conv1x1(bev)) == conv1x1(grid_sample(bev)),
so instead of materializing the two full (256,200,200) conv maps we gather
only the 4 bilinear corners of the 288 sample points from a host-transposed
HWC copy of bev_feat (channels contiguous per pixel -> 2KB indirect reads),
interpolate in 256-d, then apply the 1x1 convs to 288 vectors.

Everything else (offset MLP, DAB-style sine embeddings with on-device range
reduction, positional MLPs, 8-key-per-query attention via selection-matrix
matmuls, output projection + residual) runs on-device in fp32, feature-major
(features on partitions, queries/points on the free axis).
"""
import numpy as np

import concourse.bass as bass
import concourse.mybir as mybir
import concourse.tile as tile_mod
from concourse.bass import AP, IndirectOffsetOnAxis

F32 = mybir.dt.float32
F32R = mybir.dt.float32r
I32 = mybir.dt.int32
AF = mybir.ActivationFunctionType
OP = mybir.AluOpType

# problem constants (hardcoded per contract)
K, B, T, DIM = 6, 4, 12, 256
H, W = 200, 200
HALF = 256
G = 8                      # offset groups == sample points per query
NH = 8                     # heads
HD = 32                    # head dim of value part
NQ = 3 * T                 # queries per core = 36
NPT = NQ * G               # points per core = 288
OFFSET_SCALE = 4.0
PIX_SCALE = float(W / 102.4)          # 1.953125
PIX_BIAS = float(W / 2.0 - 0.5)       # 99.5
SCALE = 64 ** -0.5                    # 0.125
TWO_PI = float(2 * np.pi)
RC = float(3 * 2 ** 22)               # 1.5*2^23 rint magic constant
CHUNKS = [(0, 128), (128, 128), (256, 32)]   # point chunks (start, size)

# ---------------------------------------------------------------- blob layout


class Alloc:
    def __init__(self):
        self.pos = 0
        self.slices = {}

    def add(self, name, width):
        self.slices[name] = (self.pos, width)
        self.pos += width

    def __getitem__(self, name):
        return self.slices[name]


WBLOBS = {
    # most-critical first: con_q weights (smallest possible first transfer)
    "A0": [("wconq", 512), ("bconq", 2)],
    # rest of the critical path (fp32)
    "A": [("bdh", 512), ("bo1rep", 1),
          ("wo2top", 2), ("wo2bot", 2), ("bo2", 1), ("sc4pm", 2),
          ("fq2", 128), ("fk5x", 128), ("fk5y", 128), ("ident", 128)],
    # fat matmul weights (float32r)
    "R": [("wk1", 512), ("wk2", 512), ("wcat", 1024),
          ("s0", 8), ("s1", 8), ("e0", 128), ("e1", 128)],
    # the rest (fp32, needed later)
    "B": [("wq1", 512), ("bq1", 2), ("wq2", 512), ("bq2", 2),
          ("bk1", 2), ("bk2", 2), ("wout", 512), ("bout", 2)],
}


def wblob_layout(which):
    a = Alloc()
    for nm, wd in WBLOBS[which]:
        a.add(nm, wd)
    return a


NAME2BLOB = {nm: which for which, items in WBLOBS.items() for nm, _ in items}


def xblob_layout():
    a = Alloc()
    # rpx1/rpy1: rows [rp; ones] for the K=2 qse phase matmul.
    # rpo: rows [rpexp_x; rpexp_y; ones] for the K=5 kse phase matmul rhs.
    # bpm: per-chunk point-major pixel bias [sc*rpx+99.5, -sc*rpy+99.5].
    for nm, wd in [("deT", 72), ("qsT", 72), ("rpx1", 36), ("rpy1", 36),
                   ("rpo", 288), ("bpm", 6)]:
        a.add(nm, wd)
    return a


def pack_wblobs(weights):
    """weights: dict of numpy arrays (original reference layouts)."""
    lays = {w: wblob_layout(w) for w in WBLOBS}
    wbs = {w: np.zeros((128, lays[w].pos), np.float32) for w in WBLOBS}

    def put(name, arr, rows=128, coloff=0):
        lay = lays[NAME2BLOB[name]]; wb = wbs[NAME2BLOB[name]]
        s, _ = lay[name]
        wb[:rows, s + coloff: s + coloff + arr.shape[1]] = arr

    def put_mm(name, w256):  # (256, Mout) -> blocks (kc, mc) of (128, 128)
        lay = lays[NAME2BLOB[name]]; wb = wbs[NAME2BLOB[name]]
        s, _ = lay[name]
        mcs = w256.shape[1] // 128
        for kc in range(2):
            for mc in range(mcs):
                blk = w256[kc * 128:(kc + 1) * 128, mc * 128:(mc + 1) * 128]
                off = (kc * mcs + mc) * 128
                wb[:, s + off: s + off + 128] = blk

    put_mm("wconq", weights["W_con_q"])
    put("bconq", weights["b_con_q"].reshape(2, 128).T)
    # block-diag Wo1 consts: j = cc*2+h2 covers groups (2j, 2j+1)
    s, _ = lays["A"]["bdh"]
    wo1 = weights["Wo1"]  # (32, 64)
    for j in range(4):
        blk = np.zeros((128, 128), np.float32)
        blk[0:32, 0:64] = wo1 if j % 2 == 0 else 0
        if j % 2 == 0:
            blk[0:32, 0:64] = wo1
            blk[32:64, 64:128] = wo1
        else:
            blk[64:96, 0:64] = wo1
            blk[96:128, 64:128] = wo1
        wbs["A"][:, s + j * 128: s + (j + 1) * 128] = blk
    put("bo1rep", np.tile(weights["bo1"], 2)[:, None])
    wo2 = weights["Wo2"]  # (64, 2)
    top = np.zeros((128, 2), np.float32); top[0:64] = wo2
    bot = np.zeros((128, 2), np.float32); bot[64:128] = wo2
    put("wo2top", top); put("wo2bot", bot)
    put("bo2", weights["bo2"][:, None], rows=2)
    put("sc4pm", np.tile(np.array([[4 * PIX_SCALE, -4 * PIX_SCALE]],
                                  np.float32), (128, 1)))
    i64 = np.arange(128) // 2
    freq = (TWO_PI / (10000.0 ** (i64 / 64.0))).astype(np.float32)
    shift = np.where(np.arange(128) % 2 == 1, np.pi / 2, 0.0).astype(np.float32)
    fq2 = np.stack([freq, shift])                      # (2, 128)
    put("fq2", fq2, rows=2)
    fk5x = np.zeros((5, 128), np.float32)
    fk5x[0] = 4 * freq; fk5x[2] = freq; fk5x[4] = shift
    fk5y = np.zeros((5, 128), np.float32)
    fk5y[1] = 4 * freq; fk5y[3] = freq; fk5y[4] = shift
    put("fk5x", fk5x, rows=5)
    put("fk5y", fk5y, rows=5)
    put("ident", np.eye(128, dtype=np.float32))
    put_mm("wq1", weights["Wq1"]); put("bq1", weights["bq1"].reshape(2, 128).T)
    put_mm("wq2", weights["Wq2"]); put("bq2", weights["bq2"].reshape(2, 128).T)
    put_mm("wk1", weights["Wk1"]); put("bk1", weights["bk1"].reshape(2, 128).T)
    put_mm("wk2", weights["Wk2"]); put("bk2", weights["bk2"].reshape(2, 128).T)
    wcat = np.concatenate([weights["W_con_k"], weights["W_v"]], axis=1)  # (256,512)
    put_mm("wcat", wcat)
    put_mm("wout", weights["W_out"])
    put("bout", weights["b_out"].reshape(2, 128).T)
    d = np.arange(128)
    s0 = np.zeros((128, 8), np.float32)
    s0[d, d // 32] = SCALE
    s1 = np.zeros((128, 8), np.float32)
    s1[d, 4 + d // 32] = SCALE
    put("s0", s0); put("s1", s1)
    e0 = np.zeros((8, 128), np.float32)
    e0[d // 32, d] = 1.0
    e1 = np.zeros((8, 128), np.float32)
    e1[4 + d // 32, d] = 1.0
    put("e0", e0, rows=8); put("e1", e1, rows=8)
    return wbs


def pack_xblob(dec_embed, query_scale, ref_points, b, k0):
    """Per-core input blob: 36 queries = modes k0..k0+2, all T."""
    lay = xblob_layout()
    xb = np.zeros((128, lay.pos), np.float32)
    de = dec_embed[k0:k0 + 3, b].reshape(NQ, DIM)       # (36, 256)
    qs = query_scale[k0:k0 + 3, b].reshape(NQ, DIM)
    rp = ref_points[k0:k0 + 3, b].reshape(NQ, 2)

    s, _ = lay["deT"]
    xb[:, s: s + 36] = de.T[:128]
    xb[:, s + 36: s + 72] = de.T[128:]
    s, _ = lay["qsT"]
    xb[:, s: s + 36] = qs.T[:128]
    xb[:, s + 36: s + 72] = qs.T[128:]
    s, _ = lay["rpx1"]
    xb[0, s: s + 36] = rp[:, 0]
    xb[1, s: s + 36] = 1.0
    s, _ = lay["rpy1"]
    xb[0, s: s + 36] = rp[:, 1]
    xb[1, s: s + 36] = 1.0
    s, _ = lay["rpo"]
    xb[0:2, s: s + 288] = np.tile(rp.T, (1, 8))         # g-major: col = g*36+q
    xb[2, s: s + 288] = 1.0
    s, _ = lay["bpm"]
    rpe = np.tile(rp.T, (1, 8))                         # (2, 288)
    bx = PIX_SCALE * rpe[0] + PIX_BIAS
    by = -PIX_SCALE * rpe[1] + PIX_BIAS
    for c, (c0, cn) in enumerate(CHUNKS):
        xb[:cn, s + 2 * c] = bx[c0:c0 + cn]
        xb[:cn, s + 2 * c + 1] = by[c0:c0 + cn]
    return xb


# --------------------------------------------------------------- tile patches

def _split_drain_and_barrier(self, tick_clock, wait_clock):
    nc = self.nc
    drain_inst = nc.sync.drain()
    wait_clock.add_sem_waits(
        drain_inst.ins, tile_mod.ScopedClock({None: tick_clock.global_clock})
    )
    si = drain_inst.ins.sync_info
    waits = list(si.on_wait)
    if len(waits) > 1:
        si.on_wait = waits[:1]
        for i in range(1, len(waits)):
            extra = nc.sync.drain()
            extra.ins.sync_info = type(si)(on_wait=waits[i: i + 1], on_update=[])
    nc.all_engine_barrier()
    assert self.sems is not None
    popped = nc._tile_sem_poison_stack.pop()
    assert popped is self._sem_poison
    nc.clear_and_free_semaphores(list(self.sems.allocated().values()))


def split_multiwaits(nc):
    """walrus codegen supports a single sync-wait per instruction; split."""
    f = nc.m.functions[0]
    for blk in f.blocks:
        todo = [i for i in blk.instructions
                if i.sync_info is not None and len(i.sync_info.on_wait) > 1]
        for inst in todo:
            si = inst.sync_info
            waits = list(si.on_wait)
            nops = []
            for w in waits[:-1]:
                bi = nc.engines[inst.engine].nop(nofuse=True)
                ni = bi.ins
                for b2 in f.blocks:
                    if b2.instructions and b2.instructions[-1] is ni:
                        b2.instructions.pop()
                        break
                ni.sync_info = type(si)(on_wait=[w], on_update=[])
                nops.append(ni)
            si.on_wait = [waits[-1]]
            pos = blk.instructions.index(inst)
            blk.instructions[pos:pos] = nops


_PATCHED = False


def patch_tile():
    global _PATCHED
    if not _PATCHED:
        tile_mod.TileContext._drain_and_barrier = _split_drain_and_barrier
        _PATCHED = True


# ---------------------------------------------------------------- the kernel

def view3(ap, dims):
    """Build a 3D AP view on top of a 2D tile AP: dims = [[step,count],...]
    applied after the partition dim (ap.ap[0] kept)."""
    return AP(ap.tensor, ap.offset, [ap.ap[0]] + dims)


def build_nc(sim_mode=False, debug=False):
    patch_tile()
    nc = bass.Bass("TRN2")
    wlays = {w: wblob_layout(w) for w in WBLOBS}
    xlay = xblob_layout()

    # row-pair interleaved: bev[y*W+x] = [feat(y,x) (256) | feat(y+1,x) (256)]
    bev = nc.dram_tensor("bev", [H * W, 512], F32, kind="ExternalInput")
    wblA0 = nc.dram_tensor("wblA0", [128, wlays["A0"].pos], F32, kind="ExternalInput")
    wblA = nc.dram_tensor("wblA", [128, wlays["A"].pos], F32, kind="ExternalInput")
    wblR = nc.dram_tensor("wblR", [128, wlays["R"].pos], F32R, kind="ExternalInput")
    wblB = nc.dram_tensor("wblB", [128, wlays["B"].pos], F32, kind="ExternalInput")
    xbl = nc.dram_tensor("xbl", [128, xlay.pos], F32, kind="ExternalInput")
    out = nc.dram_tensor("out", [256, NQ], F32, kind="ExternalOutput")

    dbg = {}
    if debug:
        for nm, shp, dt in [
            ("d_pix", [128, 2], F32),
            ("d_idx", [128, 1], I32), ("d_sam0", [128, 256], F32),
            ("d_sim", [8, 288], F32), ("d_at", [8, 288], F32),
            ("d_kse0", [128, 288], F32R), ("d_posk0", [128, 288], F32),
            ("d_conv0", [128, 288], F32), ("d_qse0", [128, 36], F32),
            ("d_cq0", [128, 36], F32), ("d_h", [128, 144], F32),
            ("d_av0", [128, 36], F32), ("d_w40", [128, 4], F32),
        ]:
            dbg[nm] = nc.dram_tensor(nm, shp, dt, kind="ExternalOutput")

    with tile_mod.TileContext(nc) as tc:
        with (
            tc.tile_pool(name="sbuf", bufs=1) as pool,
            tc.tile_pool(name="psum", bufs=1, space="PSUM") as psum,
        ):
            # warm the {erf,tanh} activation table during the weight DMA
            wt = pool.tile([1, 1], F32)
            nc.vector.memset(wt[:], 0.0)
            warm = pool.tile([1, 1], F32)
            nc.scalar.activation(out=warm[:], in_=wt[:],
                                 func=AF.Sigmoid if sim_mode else AF.Gelu,
                                 bias=0.0)

            xb = pool.tile([128, xlay.pos], F32)
            nc.sync.dma_start(out=xb[:], in_=xbl[:])
            wbA0 = pool.tile([128, wlays["A0"].pos], F32)
            nc.sync.dma_start(out=wbA0[:], in_=wblA0[:])
            wbA = pool.tile([128, wlays["A"].pos], F32)
            nc.sync.dma_start(out=wbA[:], in_=wblA[:])
            wbR = pool.tile([128, wlays["R"].pos], F32R)
            nc.sync.dma_start(out=wbR[:], in_=wblR[:])
            wbB = pool.tile([128, wlays["B"].pos], F32)
            nc.sync.dma_start(out=wbB[:], in_=wblB[:])
            wbtiles = {"A0": wbA0, "A": wbA, "R": wbR, "B": wbB}

            def wsl(name, rows=128, off=0, width=None):
                which = NAME2BLOB[name]
                s, wd = wlays[which][name]
                if width is None:
                    width = wd - off
                return wbtiles[which][0:rows, s + off: s + off + width]

            def xsl(name, rows=128, off=0, width=None):
                s, wd = xlay[name]
                if width is None:
                    width = wd - off
                return xb[0:rows, s + off: s + off + width]

            deT = [xsl("deT", off=mc * 36, width=36) for mc in range(2)]
            qsT = [xsl("qsT", off=mc * 36, width=36) for mc in range(2)]

            # ---- 1. con_q = de @ W_con_q + b  (feature-major, 2 chunks)
            cqS = []
            for mc in range(2):
                p = psum.tile([128, 288], F32, space="PSUM", tag="psA", bufs=3, name="cqP")
                for kc in range(2):
                    nc.tensor.matmul(
                        out=p[:, :36], lhsT=wsl("wconq", off=(kc * 2 + mc) * 128, width=128),
                        rhs=deT[kc], start=(kc == 0), stop=(kc == 1))
                t = pool.tile([128, 36], F32, tag=f"cqS{mc}")
                nc.scalar.activation(out=t[:], in_=p[:, :36], func=AF.Identity,
                                     bias=wsl("bconq", off=mc, width=1))
                cqS.append(t)
            if debug:
                nc.sync.dma_start(out=dbg["d_cq0"][:], in_=cqS[0][:])

            # ---- 2. h = gelu(grouped con_q @ Wo1 + bo1): 4 block-diag mms
            hP = psum.tile([128, 288], F32, space="PSUM", tag="psA", bufs=3, name="hP")
            for j in range(4):
                cc = j // 2
                nc.tensor.matmul(
                    out=hP[:, j * 36:(j + 1) * 36],
                    lhsT=wsl("bdh", off=j * 128, width=128),
                    rhs=cqS[cc][:], start=True, stop=True)
            hS = pool.tile([128, 144], F32)
            if sim_mode:
                hx = pool.tile([128, 144], F32)
                nc.scalar.activation(out=hx[:], in_=hP[:, :144], func=AF.Identity,
                                     bias=wsl("bo1rep"))
                he = pool.tile([128, 144], F32)
                nc.scalar.activation(out=he[:], in_=hx[:], func=AF.Sigmoid,
                                     scale=float(1 / np.sqrt(2)), bias=0.0)
                nc.vector.tensor_scalar(out=he[:], in0=he[:], scalar1=0.5,
                                        scalar2=0.5, op0=OP.mult, op1=OP.add)
                nc.vector.tensor_tensor(out=hS[:], in0=hx[:], in1=he[:], op=OP.mult)
            else:
                # HW act table 'gelu' is the exact erf-based gelu
                nc.scalar.activation(out=hS[:], in_=hP[:, :144], func=AF.Gelu,
                                     bias=wsl("bo1rep"))
            if debug:
                nc.sync.dma_start(out=dbg["d_h"][:], in_=hS[:])

            # ---- 3. offsets: 2 matmuls (even/odd groups) into strided psum;
            # tanh lands in rows 0:2 of the kse-rhs tile (rows 2:5 = host
            # [rpexp_x; rpexp_y; ones]); grid math uses the tanh directly.
            offP = psum.tile([2, 288], F32, space="PSUM", tag="psA", bufs=3, name="offP")
            for m, wn in [(0, "wo2top"), (1, "wo2bot")]:
                nc.tensor.matmul(
                    out=offP[:, m * 144:(m + 1) * 144],
                    lhsT=wsl(wn, width=2),
                    rhs=hS[:], start=True, stop=True)
            kseRhs = pool.tile([5, 288], F32)
            s_rpo, _ = xlay["rpo"]
            nc.sync.dma_start(out=kseRhs[2:5, :], in_=xbl[0:3, s_rpo:s_rpo + 288])
            # tanh both halves in one op: 4D views interleave g-major cols
            kra = kseRhs[0:2, :]
            opa2 = offP[:]
            nc.scalar.activation(
                out=AP(kra.tensor, kra.offset,
                       [kra.ap[0], [72, 4], [36, 2], [1, 36]]),
                in_=AP(opa2.tensor, opa2.offset,
                       [opa2.ap[0], [36, 4], [144, 2], [1, 36]]),
                func=AF.Tanh, bias=wsl("bo2", rows=2, width=1))

            # ---- 4+5. transpose tanh to point-major, then per-point
            # geometry; gathers are issued per chunk as soon as idx is ready,
            # bilinear weights are built afterwards (off the gather path).
            idxI, w4, frs, pixdbg = [], [], [], None
            gA = []
            s_bpm, _ = xlay["bpm"]
            for c, (c0, cn) in enumerate(CHUNKS):
                tp = psum.tile([128, 2], F32, space="PSUM", tag="psA", bufs=3, name="tpP")
                nc.tensor.transpose(out=tp[:cn, :], in_=kseRhs[0:2, c0:c0 + cn],
                                    identity=wsl("ident", rows=2, width=2))
                pix = pool.tile([128, 2], F32, tag=f"pix{c}", name=f"pix{c}")
                nc.vector.tensor_tensor(out=pix[:cn, :], in0=tp[:cn, :],
                                        in1=wsl("sc4pm", rows=cn, width=2),
                                        op=OP.mult)
                nc.vector.tensor_tensor(
                    out=pix[:cn, :], in0=pix[:cn, :],
                    in1=xb[0:cn, s_bpm + 2 * c: s_bpm + 2 * c + 2], op=OP.add)
                f0 = pool.tile([128, 2], F32, tag=f"f0{c}", name=f"f0{c}")
                nc.vector.tensor_scalar(out=f0[:cn, :], in0=pix[:cn, :],
                                        scalar1=-0.5, scalar2=float(RC),
                                        op0=OP.add, op1=OP.add)
                nc.vector.tensor_scalar(out=f0[:cn, :], in0=f0[:cn, :],
                                        scalar1=float(-RC), scalar2=None,
                                        op0=OP.add)
                fr = pool.tile([128, 2], F32, tag=f"fr{c}", name=f"fr{c}")
                nc.vector.tensor_tensor(out=fr[:cn, :], in0=pix[:cn, :],
                                        in1=f0[:cn, :], op=OP.subtract)
                frs.append(fr)
                idf = pool.tile([128, 1], F32, tag=f"idf{c}", name=f"idf{c}")
                nc.vector.tensor_scalar(out=idf[:cn, :], in0=f0[:cn, 1:2],
                                        scalar1=float(W), scalar2=None,
                                        op0=OP.mult)
                nc.vector.tensor_tensor(out=idf[:cn, :], in0=idf[:cn, :],
                                        in1=f0[:cn, 0:1], op=OP.add)
                ii = pool.tile([128, 1], I32, tag=f"idxI{c}", name=f"idxI{c}")
                nc.vector.tensor_copy(out=ii[:cn, :], in_=idf[:cn, :])
                idxI.append(ii)
                ga = pool.tile([128, 1024], F32, tag=f"gA{c}", name=f"gA{c}")
                nc.gpsimd.indirect_dma_start(
                    out=ga[:cn, :], out_offset=None, in_=bev[:],
                    in_offset=IndirectOffsetOnAxis(ap=ii[:cn, :], axis=0))
                gA.append(ga)
                if debug and c == 0:
                    pixdbg = pix
            # bilinear weights (Pc, 4) = [w00, w10, w01, w11]
            for c, (c0, cn) in enumerate(CHUNKS):
                fr = frs[c]
                wxp = pool.tile([128, 2], F32, tag=f"wxp{c}", name=f"wxp{c}")
                nc.vector.tensor_scalar(out=wxp[:cn, 0:1], in0=fr[:cn, 0:1],
                                        scalar1=-1.0, scalar2=1.0,
                                        op0=OP.mult, op1=OP.add)
                nc.scalar.copy(out=wxp[:cn, 1:2], in_=fr[:cn, 0:1])
                wyp = pool.tile([128, 2], F32, tag=f"wyp{c}", name=f"wyp{c}")
                nc.vector.tensor_scalar(out=wyp[:cn, 0:1], in0=fr[:cn, 1:2],
                                        scalar1=-1.0, scalar2=1.0,
                                        op0=OP.mult, op1=OP.add)
                nc.scalar.copy(out=wyp[:cn, 1:2], in_=fr[:cn, 1:2])
                w4c = pool.tile([128, 4], F32, tag=f"w4{c}", name=f"w4{c}")
                wxa = wxp[:cn, :]
                wya = wyp[:cn, :]
                nc.vector.tensor_tensor(
                    out=view3(w4c[:cn, :], [[2, 2], [1, 2]]),
                    in0=AP(wxa.tensor, wxa.offset, [wxa.ap[0], [0, 2], [1, 2]]),
                    in1=AP(wya.tensor, wya.offset, [wya.ap[0], [1, 2], [0, 2]]),
                    op=OP.mult)
                w4.append(w4c)
            if debug:
                nc.sync.dma_start(out=dbg["d_pix"][:], in_=pixdbg[:])
                nc.sync.dma_start(out=dbg["d_idx"][:], in_=idxI[0][:])
                nc.sync.dma_start(out=dbg["d_w40"][:], in_=w4[0][:])

            # ---- helpers: sine embedding (feature-major halves)
            def sine_embed(lhs_name, lhs_rows, rhs_ap, n, tag, odt=F32):
                """phase = freq (x) meters + shift; one 128-row half."""
                ph = psum.tile([128, 288], F32, space="PSUM", tag="psA", bufs=3, name="phP")
                nc.tensor.matmul(out=ph[:, :n], lhsT=wsl(lhs_name, rows=lhs_rows),
                                 rhs=rhs_ap, start=True, stop=True)
                m1t = pool.tile([128, n], F32, tag=f"sm1{tag}")
                if n > 64:
                    nc.scalar.activation(out=m1t[:], in_=ph[:, :n], func=AF.Copy,
                                         scale=float(1.0 / TWO_PI), bias=float(RC))
                else:
                    nc.vector.tensor_scalar(out=m1t[:], in0=ph[:, :n],
                                            scalar1=float(1.0 / TWO_PI), scalar2=RC,
                                            op0=OP.mult, op1=OP.add)
                k2t = pool.tile([128, n], F32, tag=f"sk2{tag}")
                nc.vector.tensor_scalar(out=k2t[:], in0=m1t[:], scalar1=-RC,
                                        scalar2=-TWO_PI, op0=OP.add, op1=OP.mult)
                yt = pool.tile([128, n], F32, tag=f"sy{tag}")
                nc.vector.tensor_tensor(out=yt[:], in0=ph[:, :n], in1=k2t[:], op=OP.add)
                nc.vector.tensor_scalar(out=yt[:], in0=yt[:],
                                        scalar1=float(np.pi),
                                        scalar2=float(-np.pi),
                                        op0=OP.min, op1=OP.max)
                st = pool.tile([128, n], odt, tag=f"se{tag}")
                nc.scalar.activation(out=st[:], in_=yt[:], func=AF.Sin)
                return st

            def mlp2(inp2, n, wn1, bn1, wn2, bn2, tag, middt=F32):
                """two-layer MLP relu(x@W1+b1)@W2+b2, feature-major chunks."""
                mid = []
                for mc in range(2):
                    p = psum.tile([128, 288], F32, space="PSUM", tag="psA", bufs=3, name="m1P")
                    for kc in range(2):
                        nc.tensor.matmul(
                            out=p[:, :n], lhsT=wsl(wn1, off=(kc * 2 + mc) * 128, width=128),
                            rhs=inp2[kc][:], start=(kc == 0), stop=(kc == 1))
                    t = pool.tile([128, n], middt, tag=f"m1S{tag}{mc}")
                    nc.scalar.activation(out=t[:], in_=p[:, :n], func=AF.Relu,
                                         bias=wsl(bn1, off=mc, width=1))
                    mid.append(t)
                outs = []
                for mc in range(2):
                    p = psum.tile([128, 288], F32, space="PSUM", tag="psA", bufs=3, name="m2P")
                    for kc in range(2):
                        nc.tensor.matmul(
                            out=p[:, :n], lhsT=wsl(wn2, off=(kc * 2 + mc) * 128, width=128),
                            rhs=mid[kc][:], start=(kc == 0), stop=(kc == 1))
                    t = pool.tile([128, n], F32, tag=f"m2S{tag}{mc}")
                    nc.scalar.activation(out=t[:], in_=p[:, :n], func=AF.Identity,
                                         bias=wsl(bn2, off=mc, width=1))
                    outs.append(t)
                return outs

            # ---- 10a. qse sins first: their Sin triggers the trig table
            # load in the ACT-idle window before the kse sins need it.
            qse = [sine_embed("fq2", 2, xsl("rpy1", rows=2), 36, "qy"),
                   sine_embed("fq2", 2, xsl("rpx1", rows=2), 36, "qx")]
            if debug:
                nc.sync.dma_start(out=dbg["d_qse0"][:], in_=qse[0][:])
            # ---- 11. kse + pos_k (288 cols)
            kse = [sine_embed("fk5y", 5, kseRhs[:], 288, "ky", odt=F32R),
                   sine_embed("fk5x", 5, kseRhs[:], 288, "kx", odt=F32R)]
            if debug:
                nc.sync.dma_start(out=dbg["d_kse0"][:], in_=kse[0][:])
            pkS = mlp2(kse, 288, "wk1", "bk1", "wk2", "bk2", "k", middt=F32R)
            if debug:
                nc.sync.dma_start(out=dbg["d_posk0"][:], in_=pkS[0][:])
            # prefetch the exp table: queued on ACT after the Sin burst and
            # pos_k activations, well before the softmax Exp needs it.
            wt2 = pool.tile([1, 1], F32)
            nc.scalar.activation(out=wt2[:], in_=pkS[1][0:1, 0:1], func=AF.Exp)

            # ---- 7. bilinear combine -> sampled (point-major)
            # gather quarters: [c00 | c01 | c10 | c11]; weights cols
            # [w00, w10, w01, w11] -> quarter j uses w4 col [0, 2, 1, 3][j]
            sam = []
            for c, (c0, cn) in enumerate(CHUNKS):
                t1 = pool.tile([128, 256], F32, tag=f"bt1{c}", name=f"bt1{c}")
                t2 = pool.tile([128, 256], F32, tag=f"bt2{c}", name=f"bt2{c}")
                sm = pool.tile([128, 256], F32, tag=f"sam{c}", name=f"sam{c}")
                g = gA[c]
                nc.scalar.activation(out=t1[:cn, :], in_=g[:cn, 0:256],
                                     func=AF.Copy, scale=w4[c][:cn, 0:1])
                nc.vector.tensor_scalar(out=t2[:cn, :], in0=g[:cn, 256:512],
                                        scalar1=w4[c][:cn, 2:3], scalar2=None,
                                        op0=OP.mult)
                nc.vector.tensor_tensor(out=t1[:cn, :], in0=t1[:cn, :],
                                        in1=t2[:cn, :], op=OP.add)
                nc.scalar.activation(out=t2[:cn, :], in_=g[:cn, 512:768],
                                     func=AF.Copy, scale=w4[c][:cn, 1:2])
                nc.vector.tensor_tensor(out=t1[:cn, :], in0=t1[:cn, :],
                                        in1=t2[:cn, :], op=OP.add)
                nc.vector.tensor_scalar(out=t2[:cn, :], in0=g[:cn, 768:1024],
                                        scalar1=w4[c][:cn, 3:4], scalar2=None,
                                        op0=OP.mult)
                nc.vector.tensor_tensor(out=sm[:cn, :], in0=t1[:cn, :],
                                        in1=t2[:cn, :], op=OP.add)
                sam.append(sm)
            if debug:
                nc.sync.dma_start(out=dbg["d_sam0"][:], in_=sam[0][:])

            # ---- 8. transpose sampled to feature-major (256, 288) = 2 tiles
            samT = [pool.tile([128, 288], F32R, tag=f"samT{fc}", name=f"samT{fc}")
                    for fc in range(2)]
            for c, (c0, cn) in enumerate(CHUNKS):
                for fc in range(2):
                    tp = psum.tile([128, 128], F32, space="PSUM", tag="psA", bufs=3, name="samtp")
                    nc.tensor.transpose(out=tp[:, :cn],
                                        in_=sam[c][:cn, fc * 128:(fc + 1) * 128],
                                        identity=wsl("ident", rows=cn, width=cn))
                    nc.scalar.copy(out=samT[fc][:, c0:c0 + cn], in_=tp[:, :cn])

            # ---- 9. conv: con_k / v = sampled @ [W_con_k | W_v], split at the
            # g-block boundary 252 so the [0:252] part (point chunks 0+1 only)
            # runs while chunk 2's gather is still in flight.
            CR = [(0, 252), (252, 36)]
            convP = []
            vS = []
            for mc in range(4):
                p = psum.tile([128, 288], F32, space="PSUM", tag="convP", bufs=4, name="convP")
                t = (pool.tile([128, 288], F32, tag=f"vS{mc-2}", name=f"vS{mc-2}")
                     if mc >= 2 else None)
                for r0, rn in CR:
                    for kc in range(2):
                        nc.tensor.matmul(
                            out=p[:, r0:r0 + rn],
                            lhsT=wsl("wcat", off=(kc * 4 + mc) * 128, width=128),
                            rhs=samT[kc][:, r0:r0 + rn],
                            start=(kc == 0), stop=(kc == 1))
                    if t is not None:
                        nc.scalar.copy(out=t[:, r0:r0 + rn], in_=p[:, r0:r0 + rn])
                convP.append(p)
                if t is not None:
                    vS.append(t)
            if debug:
                t = pool.tile([128, 288], F32)
                nc.scalar.copy(out=t[:], in_=convP[0][:])
                nc.sync.dma_start(out=dbg["d_conv0"][:], in_=t[:])

            pqS = mlp2(qse, 36, "wq1", "bq1", "wq2", "bq2", "q")
            for mc in range(2):
                nc.vector.tensor_tensor(out=pqS[mc][:], in0=pqS[mc][:],
                                        in1=qsT[mc], op=OP.mult)

            # ---- 12. sim = scaled per-head dots via selection matmuls
            simP = psum.tile([8, 288], F32, space="PSUM", tag="simP", bufs=1, name="simP")
            pairs = [(convP[0], cqS[0], "s0"), (convP[1], cqS[1], "s1"),
                     (pkS[0], pqS[0], "s0"), (pkS[1], pqS[1], "s1")]
            tmps = [pool.tile([128, 288], F32R, tag=f"tmp{i}", name=f"tmp{i}")
                    for i in range(4)]
            for r0, rn in CR:
                ng = rn // 36
                for i, (kpart, qpart, sname) in enumerate(pairs):
                    tmp = tmps[i]
                    qap = qpart[:]
                    ka = kpart[:]
                    ta = tmp[:]
                    nc.vector.tensor_tensor(
                        out=AP(ta.tensor, ta.offset + r0, [ta.ap[0], [36, ng], [1, 36]]),
                        in0=AP(ka.tensor, ka.offset + r0, [ka.ap[0], [36, ng], [1, 36]]),
                        in1=AP(qap.tensor, qap.offset, [qap.ap[0], [0, ng], [1, 36]]),
                        op=OP.mult)
                    nc.tensor.matmul(out=simP[:, r0:r0 + rn], lhsT=wsl(sname, width=8),
                                     rhs=tmp[:, r0:r0 + rn],
                                     start=(i == 0), stop=(i == 3))
            if debug:
                t = pool.tile([8, 288], F32)
                nc.vector.tensor_copy(out=t[:], in_=simP[:])
                nc.sync.dma_start(out=dbg["d_sim"][:], in_=t[:])

            # ---- 13+14. softmax (deferred normalization) + weighted values.
            # exp -> unnormalized attn; head-expand exp and the per-(h,q)
            # reciprocal separately, normalize the reduced (128, 36) output.
            # (|sim| <= ~3 so exp without max-subtract is safe; softmax is
            # shift-invariant so the result is identical.)
            ex = pool.tile([8, 288], F32R)
            nc.scalar.activation(out=ex[:], in_=simP[:], func=AF.Exp)
            sm = pool.tile([8, 36], F32)
            nc.vector.reduce_sum(out=sm[:], in_=view3(ex[:], [[1, 36], [36, 8]]),
                                 axis=mybir.AxisListType.X)
            rc = pool.tile([8, 36], F32R)
            with nc.allow_low_precision(reason="f32r keeps full fp32 range; "
                                        "mantissa rounding is ~1e-4 rel"):
                nc.vector.reciprocal(out=rc[:], in_=sm[:])
            if debug:
                at = pool.tile([8, 288], F32)
                rca = rc[:]
                nc.vector.tensor_tensor(
                    out=view3(at[:], [[1, 36], [36, 8]]),
                    in0=view3(ex[:], [[1, 36], [36, 8]]),
                    in1=AP(rca.tensor, rca.offset, [rca.ap[0], [1, 36], [0, 8]]),
                    op=OP.mult)
                nc.sync.dma_start(out=dbg["d_at"][:], in_=at[:])

            avT = []
            for fc in range(2):
                ae = psum.tile([128, 288], F32, space="PSUM", tag="psA", bufs=3, name="aeP")
                nc.tensor.matmul(out=ae[:], lhsT=wsl(f"e{fc}", rows=8, width=128),
                                 rhs=ex[:], start=True, stop=True)
                pr = pool.tile([128, 288], F32, tag=f"pr{fc}", name=f"pr{fc}")
                nc.vector.tensor_tensor(out=pr[:], in0=vS[fc][:], in1=ae[:],
                                        op=OP.mult)
                avu = pool.tile([128, 36], F32, tag=f"avu{fc}", name=f"avu{fc}")
                nc.vector.reduce_sum(out=avu[:], in_=view3(pr[:], [[1, 36], [36, 8]]),
                                     axis=mybir.AxisListType.X)
                re = psum.tile([128, 288], F32, space="PSUM", tag="psA", bufs=3, name="reP")
                nc.tensor.matmul(out=re[:, :36], lhsT=wsl(f"e{fc}", rows=8, width=128),
                                 rhs=rc[:], start=True, stop=True)
                av = pool.tile([128, 36], F32, tag=f"avT{fc}", name=f"avT{fc}")
                nc.vector.tensor_tensor(out=av[:], in0=avu[:], in1=re[:, :36],
                                        op=OP.mult)
                avT.append(av)
            if debug:
                nc.sync.dma_start(out=dbg["d_av0"][:], in_=avT[0][:])

            # ---- 15. out = attn_out @ W_out + b_out + identity (single DMA)
            oT = pool.tile([128, 72], F32)
            for mc in range(2):
                p = psum.tile([128, 288], F32, space="PSUM", tag="psA", bufs=3, name="oP")
                for kc in range(2):
                    nc.tensor.matmul(
                        out=p[:, :36], lhsT=wsl("wout", off=(kc * 2 + mc) * 128, width=128),
                        rhs=avT[kc][:], start=(kc == 0), stop=(kc == 1))
                nc.scalar.activation(out=oT[:, mc * 36:(mc + 1) * 36],
                                     in_=p[:, :36], func=AF.Identity,
                                     bias=wsl("bout", off=mc, width=1))
                nc.vector.tensor_tensor(out=oT[:, mc * 36:(mc + 1) * 36],
                                        in0=oT[:, mc * 36:(mc + 1) * 36],
                                        in1=deT[mc], op=OP.add)
            ota = oT[:]
            oda = out[:]
            nc.sync.dma_start(
                out=AP(oda.tensor, oda.offset, [[36, 128], [128 * 36, 2], [1, 36]]),
                in_=AP(ota.tensor, ota.offset, [[72, 128], [36, 2], [1, 36]]))

    return nc


# ------------------------------------------------------------------- driver

def make_in_maps(dec_embed, bev_feat, query_scale, ref_points, weights):
    wbs = pack_wblobs(weights)
    in_maps = []
    for c in range(8):
        b, kh = c // 2, c % 2
        hwc = bev_feat[b].transpose(1, 2, 0).reshape(H * W, 256)
        bev_hwc = np.zeros((H * W, 512), np.float32)
        bev_hwc[:, 0:256] = hwc
        bev_hwc[:(H - 1) * W, 256:512] = hwc[W:]
        bev_hwc = np.ascontiguousarray(bev_hwc)
        xb = pack_xblob(dec_embed, query_scale, ref_points, b, 3 * kh)
        in_maps.append({"bev": bev_hwc, "wblA0": wbs["A0"], "wblA": wbs["A"],
                        "wblR": wbs["R"], "wblB": wbs["B"], "xbl": xb})
    return in_maps


def assemble_output(results, dec_dtype=np.float32):
    out = np.zeros((K, B, T, DIM), np.float32)
    for c in range(8):
        b, kh = c // 2, c % 2
        oc = results[c]["out"]                     # (256, 36)
        out[3 * kh:3 * kh + 3, b] = oc.T.reshape(3, T, DIM)
    return out


_WNAMES = ["W_con_q", "b_con_q", "W_con_k", "W_v", "Wq1", "bq1", "Wq2", "bq2",
           "Wk1", "bk1", "Wk2", "bk2", "Wo1", "bo1", "Wo2", "bo2",
           "W_out", "b_out"]


def kernel(**inputs):
    from concourse.bass_utils import run_bass_kernel_spmd
    dec_embed = np.asarray(inputs["dec_embed"], np.float32)
    bev_feat = np.asarray(inputs["bev_feat"], np.float32)
    query_scale = np.asarray(inputs["query_scale"], np.float32)
    ref_points = np.asarray(inputs["ref_points"], np.float32)
    weights = {n: np.asarray(inputs[n], np.float32) for n in _WNAMES}

    nc = build_nc(sim_mode=False, debug=False)
    split_multiwaits(nc)
    in_maps = make_in_maps(dec_embed, bev_feat, query_scale, ref_points, weights)
    res = run_bass_kernel_spmd(nc, in_maps, list(range(8)))
    return assemble_output(res.results)



# revision 9
# speedup vs baseline: 1.3379x; 1.3379x over previous
"""BEV deformable cross-attention kernel for 8 Trainium2 NeuronCores.

Strategy (per core): data-parallel over (B x K-half): core c handles batch
b = c//2 and modes k in {3*(c%2) .. +3}, i.e. 36 queries, 288 sample points.

Key algebraic move: grid_sample(conv1x1(bev)) == conv1x1(grid_sample(bev)),
so instead of materializing the two full (256,200,200) conv maps we gather
only the 4 bilinear corners of the 288 sample points from a host-transposed
HWC copy of bev_feat (channels contiguous per pixel -> 2KB indirect reads),
interpolate in 256-d, then apply the 1x1 convs to 288 vectors.

v2 perf notes vs the first working version:
- all fat matmuls run in bf16 (fp32 matmuls lower to 2 half-rate HW passes);
  only the sine-phase matmuls and the geometry path stay fp32.
- weights/inputs land via parallel DMA queues (gpsimd + sync) so the first
  matmul no longer waits on unrelated blobs.
- gelu is computed through the Silu table (gelu(x) ~ x*sigmoid(1.702x),
  exact for the tiny pre-activations here) so {silu,tanh,sin} share one
  activation table and only one mid-kernel table switch (exp) remains.
- the bilinear combine is 4 fused scalar_tensor_tensor ops per chunk on the
  Pool engine; softmax normalizes before head-expansion (no re-expand mm).
"""
import numpy as np
import ml_dtypes

import concourse.bass as bass
import concourse.mybir as mybir
import concourse.tile as tile_mod
from concourse.bass import AP, IndirectOffsetOnAxis

F32 = mybir.dt.float32
BF16 = mybir.dt.bfloat16
I32 = mybir.dt.int32
AF = mybir.ActivationFunctionType
OP = mybir.AluOpType
NPBF = ml_dtypes.bfloat16

# problem constants (hardcoded per contract)
K, B, T, DIM = 6, 4, 12, 256
H, W = 200, 200
HALF = 256
G = 8                      # offset groups == sample points per query
NQ = 3 * T                 # queries per core = 36
NPT = NQ * G               # points per core = 288
OFFSET_SCALE = 4.0
PIX_SCALE = float(W / 102.4)          # 1.953125
PIX_BIAS = float(W / 2.0 - 0.5)       # 99.5
SCALE = 64 ** -0.5                    # 0.125
TWO_PI = float(2 * np.pi)
RC = float(3 * 2 ** 22)               # 1.5*2^23 rint magic constant
SILU_A = 1.702                        # gelu(x) ~ silu(1.702 x)/1.702
CHUNKS = [(0, 128), (128, 128), (256, 32)]   # point chunks (start, size)

# ---------------------------------------------------------------- blob layout


class Alloc:
    def __init__(self):
        self.pos = 0
        self.slices = {}

    def add(self, name, width):
        self.slices[name] = (self.pos, width)
        self.pos += width

    def __getitem__(self, name):
        return self.slices[name]


# bf16 matmul-weight blob; split points F0/F1/F2 are separate DMAs so the
# critical-path prefix (wconq) lands first.
WF_ITEMS = [("wconq", 512),                                        # F0
            ("bdh", 512), ("wo2t", 2), ("wo2b", 2),                # F1...
            ("wq1", 512), ("wq2", 512),
            ("s0", 8), ("s1", 8), ("e0", 128), ("e1", 128), ("identB", 128),
            ("wk1", 512), ("wk2", 512), ("wcat", 1024), ("wout", 512)]  # F2
F0_END = 512
F1_END = 512 + 512 + 4 + 1024 + 16 + 256 + 128     # 2452

# fp32 misc blob: biases (as (128,2) column pairs), geometry consts,
# sine-phase weights.
WG_ITEMS = [("bconq", 2), ("bo1s", 1), ("bo2", 1), ("sc4pm", 2),
            ("fq2", 128), ("fk5x", 128), ("fk5y", 128), ("id2", 2),
            ("bq1", 2), ("bq2", 2), ("bk1", 2), ("bk2", 2), ("bout", 2)]

# fp32 per-core input blob. rpo holds [tanh_x; tanh_y; rpx; rpy; ones] rows:
# partitions 0:2 are blank (filled by the on-device tanh), 2:5 host data.
XB_ITEMS = [("deT", 72), ("qsT", 72), ("rpyx1", 72), ("rpo", 288), ("bpm", 6)]


def _layout(items):
    a = Alloc()
    for nm, wd in items:
        a.add(nm, wd)
    return a


WF_LAY = _layout(WF_ITEMS)
WG_LAY = _layout(WG_ITEMS)
XB_LAY = _layout(XB_ITEMS)


def _put_mm(dst, lay, name, w256):
    """(256, Mout) -> (kc, mc) blocks of (128, 128) at s + (kc*mcs+mc)*128."""
    s, _ = lay[name]
    mcs = w256.shape[1] // 128
    for kc in range(2):
        for mc in range(mcs):
            blk = w256[kc * 128:(kc + 1) * 128, mc * 128:(mc + 1) * 128]
            off = (kc * mcs + mc) * 128
            dst[:, s + off: s + off + 128] = blk


def pack_wf(weights):
    wf = np.zeros((128, WF_LAY.pos), np.float32)
    lay = WF_LAY

    def put(name, arr, rows=128):
        s, _ = lay[name]
        wf[:rows, s: s + arr.shape[1]] = arr

    _put_mm(wf, lay, "wconq", weights["W_con_q"])
    # block-diag Wo1: block j covers groups (2j, 2j+1); even j from feature
    # chunk 0 rows, odd j from chunk 1 rows.
    s, _ = lay["bdh"]
    wo1 = weights["Wo1"]  # (32, 64)
    for j in range(4):
        blk = np.zeros((128, 128), np.float32)
        if j % 2 == 0:
            blk[0:32, 0:64] = wo1
            blk[32:64, 64:128] = wo1
        else:
            blk[64:96, 0:64] = wo1
            blk[96:128, 64:128] = wo1
        wf[:, s + j * 128: s + (j + 1) * 128] = blk
    wo2 = weights["Wo2"] / SILU_A          # undo the silu input scale
    top = np.zeros((128, 2), np.float32); top[0:64] = wo2
    bot = np.zeros((128, 2), np.float32); bot[64:128] = wo2
    put("wo2t", top); put("wo2b", bot)
    _put_mm(wf, lay, "wq1", weights["Wq1"])
    _put_mm(wf, lay, "wq2", weights["Wq2"])
    _put_mm(wf, lay, "wk1", weights["Wk1"])
    _put_mm(wf, lay, "wk2", weights["Wk2"])
    wcat = np.concatenate([weights["W_con_k"], weights["W_v"]], axis=1)
    _put_mm(wf, lay, "wcat", wcat)
    _put_mm(wf, lay, "wout", weights["W_out"])
    d = np.arange(128)
    s0 = np.zeros((128, 8), np.float32); s0[d, d // 32] = SCALE
    s1 = np.zeros((128, 8), np.float32); s1[d, 4 + d // 32] = SCALE
    put("s0", s0); put("s1", s1)
    e0 = np.zeros((8, 128), np.float32); e0[d // 32, d] = 1.0
    e1 = np.zeros((8, 128), np.float32); e1[4 + d // 32, d] = 1.0
    put("e0", e0, rows=8); put("e1", e1, rows=8)
    put("identB", np.eye(128, dtype=np.float32))
    return wf.astype(NPBF)


def _freq_shift():
    i64 = np.arange(128) // 2
    freq = (TWO_PI / (10000.0 ** (i64 / 64.0))).astype(np.float32)
    shift = np.where(np.arange(128) % 2 == 1, np.pi / 2, 0.0).astype(np.float32)
    return freq, shift


def pack_wg(weights):
    wg = np.zeros((128, WG_LAY.pos), np.float32)
    lay = WG_LAY

    def put(name, arr, rows=128):
        s, _ = lay[name]
        wg[:rows, s: s + arr.shape[1]] = arr

    put("bconq", weights["b_con_q"].reshape(2, 128).T)
    put("bo1s", SILU_A * np.tile(weights["bo1"], 2)[:, None])
    put("bo2", weights["bo2"][:, None], rows=2)
    put("sc4pm", np.tile(np.array([[4 * PIX_SCALE, -4 * PIX_SCALE]],
                                  np.float32), (128, 1)))
    freq, shift = _freq_shift()
    put("fq2", np.stack([freq, shift]), rows=2)
    fk5x = np.zeros((5, 128), np.float32)
    fk5x[0] = 4 * freq; fk5x[2] = freq; fk5x[4] = shift
    fk5y = np.zeros((5, 128), np.float32)
    fk5y[1] = 4 * freq; fk5y[3] = freq; fk5y[4] = shift
    put("fk5x", fk5x, rows=5)
    put("fk5y", fk5y, rows=5)
    put("id2", np.eye(2, dtype=np.float32), rows=2)
    put("bq1", weights["bq1"].reshape(2, 128).T)
    put("bq2", weights["bq2"].reshape(2, 128).T)
    put("bk1", weights["bk1"].reshape(2, 128).T)
    put("bk2", weights["bk2"].reshape(2, 128).T)
    put("bout", weights["b_out"].reshape(2, 128).T)
    return wg


def pack_xb(dec_embed, query_scale, ref_points, b, k0):
    lay = XB_LAY
    xb = np.zeros((128, lay.pos), np.float32)
    de = dec_embed[k0:k0 + 3, b].reshape(NQ, DIM)       # (36, 256)
    qs = query_scale[k0:k0 + 3, b].reshape(NQ, DIM)
    rp = ref_points[k0:k0 + 3, b].reshape(NQ, 2)

    s, _ = lay["deT"]
    xb[:, s: s + 36] = de.T[:128]
    xb[:, s + 36: s + 72] = de.T[128:]
    s, _ = lay["qsT"]
    xb[:, s: s + 36] = qs.T[:128]
    xb[:, s + 36: s + 72] = qs.T[128:]
    s, _ = lay["rpyx1"]
    xb[0, s: s + 36] = rp[:, 1]                         # y first (DAB order)
    xb[0, s + 36: s + 72] = rp[:, 0]
    xb[1, s: s + 72] = 1.0
    s, _ = lay["rpo"]
    rpe = np.tile(rp.T, (1, 8))                         # g-major: col = g*36+q
    xb[2, s: s + 288] = rpe[0]
    xb[3, s: s + 288] = rpe[1]
    xb[4, s: s + 288] = 1.0
    s, _ = lay["bpm"]
    bx = PIX_SCALE * rpe[0] + PIX_BIAS
    by = -PIX_SCALE * rpe[1] + PIX_BIAS
    for c, (c0, cn) in enumerate(CHUNKS):
        xb[:cn, s + 2 * c] = bx[c0:c0 + cn]
        xb[:cn, s + 2 * c + 1] = by[c0:c0 + cn]
    return xb


def pack_xh(dec_embed, b, k0):
    de = dec_embed[k0:k0 + 3, b].reshape(NQ, DIM)
    xh = np.zeros((128, 72), np.float32)
    xh[:, 0:36] = de.T[:128]
    xh[:, 36:72] = de.T[128:]
    return xh.astype(NPBF)


# --------------------------------------------------------------- tile patches

def _split_drain_and_barrier(self, tick_clock, wait_clock):
    nc = self.nc
    drain_inst = nc.sync.drain()
    wait_clock.add_sem_waits(
        drain_inst.ins, tile_mod.ScopedClock({None: tick_clock.global_clock})
    )
    si = drain_inst.ins.sync_info
    waits = list(si.on_wait)
    if len(waits) > 1:
        si.on_wait = waits[:1]
        for i in range(1, len(waits)):
            extra = nc.sync.drain()
            extra.ins.sync_info = type(si)(on_wait=waits[i: i + 1], on_update=[])
    nc.all_engine_barrier()
    assert self.sems is not None
    popped = nc._tile_sem_poison_stack.pop()
    assert popped is self._sem_poison
    nc.clear_and_free_semaphores(list(self.sems.allocated().values()))


def split_multiwaits(nc):
    """walrus codegen supports a single sync-wait per instruction; split."""
    f = nc.m.functions[0]
    for blk in f.blocks:
        todo = [i for i in blk.instructions
                if i.sync_info is not None and len(i.sync_info.on_wait) > 1]
        for inst in todo:
            si = inst.sync_info
            waits = list(si.on_wait)
            nops = []
            for w in waits[:-1]:
                bi = nc.engines[inst.engine].nop(nofuse=True)
                ni = bi.ins
                for b2 in f.blocks:
                    if b2.instructions and b2.instructions[-1] is ni:
                        b2.instructions.pop()
                        break
                ni.sync_info = type(si)(on_wait=[w], on_update=[])
                nops.append(ni)
            si.on_wait = [waits[-1]]
            pos = blk.instructions.index(inst)
            blk.instructions[pos:pos] = nops


_PATCHED = False


def patch_tile():
    global _PATCHED
    if not _PATCHED:
        tile_mod.TileContext._drain_and_barrier = _split_drain_and_barrier
        _PATCHED = True


# ---------------------------------------------------------------- the kernel

def view3(ap, dims):
    """3D AP view over a 2D tile AP: dims = [[step,count],...] after ap[0]."""
    return AP(ap.tensor, ap.offset, [ap.ap[0]] + dims)


def build_nc(sim_mode=False, debug=False):
    patch_tile()
    nc = bass.Bass("TRN2")

    # row-pair interleaved bf16: bev[y*W+x] = [feat(y,x) | feat(y+1,x)]
    bev = nc.dram_tensor("bev", [H * W, 512], BF16, kind="ExternalInput")
    wfD = nc.dram_tensor("wf", [128, WF_LAY.pos], BF16, kind="ExternalInput")
    wgD = nc.dram_tensor("wg", [128, WG_LAY.pos], F32, kind="ExternalInput")
    xbD = nc.dram_tensor("xbl", [128, XB_LAY.pos], F32, kind="ExternalInput")
    xhD = nc.dram_tensor("xh", [128, 72], BF16, kind="ExternalInput")
    out = nc.dram_tensor("out", [256, NQ], F32, kind="ExternalOutput")

    dbg = {}
    if debug:
        for nm, shp, dt in [
            ("d_pix", [128, 2], F32), ("d_idx", [128, 1], I32),
            ("d_w40", [128, 4], F32), ("d_sam0", [128, 256], BF16),
            ("d_cq0", [128, 36], BF16), ("d_h", [128, 144], BF16),
            ("d_qse0", [128, 36], BF16), ("d_kse0", [128, 288], BF16),
            ("d_posk0", [128, 288], BF16), ("d_conv0", [128, 288], F32),
            ("d_sim", [8, 288], F32), ("d_at", [8, 288], BF16),
            ("d_av0", [128, 36], BF16),
        ]:
            dbg[nm] = nc.dram_tensor(nm, shp, dt, kind="ExternalOutput")

    with tile_mod.TileContext(nc) as tc:
        with (
            tc.tile_pool(name="sbuf", bufs=1) as pool,
            tc.tile_pool(name="psum", bufs=1, space="PSUM") as psum,
        ):
            # warm the {silu,tanh,sin} table during the input DMAs
            wt = pool.tile([1, 1], F32)
            nc.vector.memset(wt[:], 0.0)
            warm = pool.tile([1, 1], F32)
            nc.scalar.activation(out=warm[:], in_=wt[:],
                                 func=AF.Sigmoid if sim_mode else AF.Silu,
                                 bias=0.0)

            # ---- input DMAs: critical prefix on gpsimd (cheap dispatch),
            # the rest on sync, so the first matmul waits only on xh+F0.
            xh = pool.tile([128, 72], BF16)
            nc.gpsimd.dma_start(out=xh[:], in_=xhD[:])
            wf = pool.tile([128, WF_LAY.pos], BF16)
            nc.gpsimd.dma_start(out=wf[:, 0:F0_END], in_=wfD[:, 0:F0_END])
            wg = pool.tile([128, WG_LAY.pos], F32)
            nc.sync.dma_start(out=wg[:], in_=wgD[:])
            xb = pool.tile([128, XB_LAY.pos], F32)
            nc.sync.dma_start(out=xb[:], in_=xbD[:])
            nc.gpsimd.dma_start(out=wf[:, F0_END:F1_END],
                                in_=wfD[:, F0_END:F1_END])
            nc.gpsimd.dma_start(out=wf[:, F1_END:WF_LAY.pos],
                                in_=wfD[:, F1_END:WF_LAY.pos])

            def wfs(name, rows=128, off=0, width=None):
                s, wd = WF_LAY[name]
                if width is None:
                    width = wd - off
                return wf[0:rows, s + off: s + off + width]

            def wgs(name, rows=128, off=0, width=None):
                s, wd = WG_LAY[name]
                if width is None:
                    width = wd - off
                return wg[0:rows, s + off: s + off + width]

            def xbs(name, rows=128, off=0, width=None):
                s, wd = XB_LAY[name]
                if width is None:
                    width = wd - off
                return xb[0:rows, s + off: s + off + width]

            # ---- 1. con_q = de @ W_con_q + b   (bf16, one (128,72) psum)
            cqP = psum.tile([128, 288], F32, space="PSUM", tag="psA", bufs=3,
                            name="cqP")
            for mc in range(2):
                for kc in range(2):
                    nc.tensor.matmul(
                        out=cqP[:, mc * 36:(mc + 1) * 36],
                        lhsT=wfs("wconq", off=(kc * 2 + mc) * 128, width=128),
                        rhs=xh[:, kc * 36:(kc + 1) * 36],
                        start=(kc == 0), stop=(kc == 1))
            cqS = pool.tile([128, 72], BF16, name="cqS")
            for mc in range(2):
                nc.scalar.activation(out=cqS[:, mc * 36:(mc + 1) * 36],
                                     in_=cqP[:, mc * 36:(mc + 1) * 36],
                                     func=AF.Identity,
                                     bias=wgs("bconq", off=mc, width=1))
            if debug:
                nc.sync.dma_start(out=dbg["d_cq0"][:], in_=cqS[:, 0:36])

            # ---- 2. h = gelu(grouped con_q @ Wo1 + bo1) via silu table
            hP = psum.tile([128, 288], F32, space="PSUM", tag="psA", bufs=3,
                           name="hP")
            for j in range(4):
                cc = j // 2
                nc.tensor.matmul(
                    out=hP[:, j * 36:(j + 1) * 36],
                    lhsT=wfs("bdh", off=j * 128, width=128),
                    rhs=cqS[:, cc * 36:(cc + 1) * 36], start=True, stop=True)
            hS = pool.tile([128, 144], BF16, name="hS")
            if sim_mode:
                hx = pool.tile([128, 144], F32)
                nc.scalar.activation(out=hx[:], in_=hP[:, :144],
                                     func=AF.Identity, scale=SILU_A,
                                     bias=wgs("bo1s"))
                he = pool.tile([128, 144], F32)
                nc.scalar.activation(out=he[:], in_=hx[:], func=AF.Sigmoid,
                                     bias=0.0)
                nc.vector.tensor_tensor(out=hS[:], in0=hx[:], in1=he[:],
                                        op=OP.mult)
            else:
                nc.scalar.activation(out=hS[:], in_=hP[:, :144], func=AF.Silu,
                                     scale=SILU_A, bias=wgs("bo1s"))
            if debug:
                nc.sync.dma_start(out=dbg["d_h"][:], in_=hS[:])

            # ---- 3. offsets: 2 bf16 matmuls; tanh lands in xb rows 0:2 of
            # the rpo region (rows 2:5 are host [rpx; rpy; ones]).
            offP = psum.tile([2, 288], F32, space="PSUM", tag="psA", bufs=3,
                             name="offP")
            for m, wn in [(0, "wo2t"), (1, "wo2b")]:
                nc.tensor.matmul(
                    out=offP[:, m * 144:(m + 1) * 144],
                    lhsT=wfs(wn, width=2), rhs=hS[:], start=True, stop=True)
            s_rpo, _ = XB_LAY["rpo"]
            kra = xb[0:2, s_rpo:s_rpo + 288]
            opa = offP[:]
            nc.scalar.activation(
                out=AP(kra.tensor, kra.offset,
                       [kra.ap[0], [72, 4], [36, 2], [1, 36]]),
                in_=AP(opa.tensor, opa.offset,
                       [opa.ap[0], [36, 4], [144, 2], [1, 36]]),
                func=AF.Tanh, bias=wgs("bo2", rows=2, width=1))
            kseRhs = xb[0:5, s_rpo:s_rpo + 288]

            # ---- 4. per-chunk geometry -> indices -> gathers (bf16 rows)
            s_bpm, _ = XB_LAY["bpm"]
            frs, idxI, gA, w4 = [], [], [], []
            pix0 = None
            for c, (c0, cn) in enumerate(CHUNKS):
                tp = psum.tile([128, 2], F32, space="PSUM", tag="psA", bufs=3,
                               name=f"tpP{c}")
                nc.tensor.transpose(out=tp[:cn, :], in_=kseRhs[0:2, c0:c0 + cn],
                                    identity=wgs("id2", rows=2, width=2))
                pix = pool.tile([128, 2], F32, name=f"pix{c}")
                if c == 0:
                    pix0 = pix
                nc.vector.tensor_tensor(out=pix[:cn, :], in0=tp[:cn, :],
                                        in1=wgs("sc4pm", rows=cn, width=2),
                                        op=OP.mult)
                nc.vector.tensor_tensor(
                    out=pix[:cn, :], in0=pix[:cn, :],
                    in1=xb[0:cn, s_bpm + 2 * c: s_bpm + 2 * c + 2], op=OP.add)
                f0 = pool.tile([128, 2], F32, name=f"f0{c}")
                nc.vector.tensor_scalar(out=f0[:cn, :], in0=pix[:cn, :],
                                        scalar1=-0.5, scalar2=float(RC),
                                        op0=OP.add, op1=OP.add)
                nc.vector.tensor_scalar(out=f0[:cn, :], in0=f0[:cn, :],
                                        scalar1=float(-RC), scalar2=None,
                                        op0=OP.add)
                fr = pool.tile([128, 2], F32, name=f"fr{c}")
                nc.vector.tensor_tensor(out=fr[:cn, :], in0=pix[:cn, :],
                                        in1=f0[:cn, :], op=OP.subtract)
                frs.append(fr)
                idf = pool.tile([128, 1], F32, name=f"idf{c}")
                nc.vector.scalar_tensor_tensor(
                    out=idf[:cn, :], in0=f0[:cn, 1:2], scalar=float(W),
                    in1=f0[:cn, 0:1], op0=OP.mult, op1=OP.add)
                ii = pool.tile([128, 1], I32, name=f"idxI{c}")
                nc.vector.tensor_copy(out=ii[:cn, :], in_=idf[:cn, :])
                idxI.append(ii)
                ga = pool.tile([128, 1024], BF16, name=f"gA{c}")
                nc.gpsimd.indirect_dma_start(
                    out=ga[:cn, :], out_offset=None, in_=bev[:],
                    in_offset=IndirectOffsetOnAxis(ap=ii[:cn, :], axis=0))
                gA.append(ga)
            # bilinear weights (Pc, 4) = [w00, w10, w01, w11]
            for c, (c0, cn) in enumerate(CHUNKS):
                fr = frs[c]
                wxp = pool.tile([128, 2], F32, name=f"wxp{c}")
                nc.vector.tensor_scalar(out=wxp[:cn, 0:1], in0=fr[:cn, 0:1],
                                        scalar1=-1.0, scalar2=1.0,
                                        op0=OP.mult, op1=OP.add)
                nc.vector.tensor_copy(out=wxp[:cn, 1:2], in_=fr[:cn, 0:1])
                wyp = pool.tile([128, 2], F32, name=f"wyp{c}")
                nc.vector.tensor_scalar(out=wyp[:cn, 0:1], in0=fr[:cn, 1:2],
                                        scalar1=-1.0, scalar2=1.0,
                                        op0=OP.mult, op1=OP.add)
                nc.vector.tensor_copy(out=wyp[:cn, 1:2], in_=fr[:cn, 1:2])
                w4c = pool.tile([128, 4], F32, name=f"w4{c}")
                wxa = wxp[:cn, :]
                wya = wyp[:cn, :]
                nc.vector.tensor_tensor(
                    out=view3(w4c[:cn, :], [[2, 2], [1, 2]]),
                    in0=AP(wxa.tensor, wxa.offset, [wxa.ap[0], [0, 2], [1, 2]]),
                    in1=AP(wya.tensor, wya.offset, [wya.ap[0], [1, 2], [0, 2]]),
                    op=OP.mult)
                w4.append(w4c)
            if debug:
                nc.sync.dma_start(out=dbg["d_pix"][:], in_=pix0[:])
                nc.sync.dma_start(out=dbg["d_idx"][:], in_=idxI[0][:])
                nc.sync.dma_start(out=dbg["d_w40"][:], in_=w4[0][:])

            # ---- 5. qse (y|x merged): one fp32 phase matmul + one sin chain
            phQ = psum.tile([128, 288], F32, space="PSUM", tag="psA", bufs=3,
                            name="phQ")
            nc.tensor.matmul(out=phQ[:, :72], lhsT=wgs("fq2", rows=2),
                             rhs=xbs("rpyx1", rows=2), start=True, stop=True)
            qse = pool.tile([128, 72], BF16, name="qse")

            def sin_reduce(ph_ap, n, out_ap, tag):
                m1 = pool.tile([128, n], F32, name=f"m1{tag}")
                nc.vector.tensor_scalar(out=m1[:], in0=ph_ap,
                                        scalar1=float(1.0 / TWO_PI),
                                        scalar2=RC, op0=OP.mult, op1=OP.add)
                nc.vector.tensor_scalar(out=m1[:], in0=m1[:], scalar1=-RC,
                                        scalar2=-TWO_PI, op0=OP.add,
                                        op1=OP.mult)
                yt = pool.tile([128, n], F32, name=f"yt{tag}")
                nc.vector.tensor_tensor(out=yt[:], in0=ph_ap, in1=m1[:],
                                        op=OP.add)
                nc.vector.tensor_scalar(out=yt[:], in0=yt[:],
                                        scalar1=float(np.pi),
                                        scalar2=float(-np.pi),
                                        op0=OP.min, op1=OP.max)
                nc.scalar.activation(out=out_ap, in_=yt[:], func=AF.Sin)

            sin_reduce(phQ[:, :72], 72, qse[:], "q")
            if debug:
                nc.sync.dma_start(out=dbg["d_qse0"][:], in_=qse[:, 0:36])

            # ---- 6. pos_q MLP (bf16, early: overlaps the gathers)
            mqP = psum.tile([128, 288], F32, space="PSUM", tag="psA", bufs=3,
                            name="mqP")
            for mc in range(2):
                for kc in range(2):
                    nc.tensor.matmul(
                        out=mqP[:, mc * 36:(mc + 1) * 36],
                        lhsT=wfs("wq1", off=(kc * 2 + mc) * 128, width=128),
                        rhs=qse[:, kc * 36:(kc + 1) * 36],
                        start=(kc == 0), stop=(kc == 1))
            midQ = pool.tile([128, 72], BF16, name="midQ")
            for mc in range(2):
                nc.vector.tensor_scalar(
                    out=midQ[:, mc * 36:(mc + 1) * 36],
                    in0=mqP[:, mc * 36:(mc + 1) * 36],
                    scalar1=wgs("bq1", off=mc, width=1), scalar2=0.0,
                    op0=OP.add, op1=OP.max)
            pqP = psum.tile([128, 288], F32, space="PSUM", tag="psA", bufs=3,
                            name="pqP")
            for mc in range(2):
                for kc in range(2):
                    nc.tensor.matmul(
                        out=pqP[:, mc * 36:(mc + 1) * 36],
                        lhsT=wfs("wq2", off=(kc * 2 + mc) * 128, width=128),
                        rhs=midQ[:, kc * 36:(kc + 1) * 36],
                        start=(kc == 0), stop=(kc == 1))
            pqS = pool.tile([128, 72], BF16, name="pqS")
            for mc in range(2):
                nc.vector.scalar_tensor_tensor(
                    out=pqS[:, mc * 36:(mc + 1) * 36],
                    in0=pqP[:, mc * 36:(mc + 1) * 36],
                    scalar=wgs("bq2", off=mc, width=1),
                    in1=xbs("qsT", off=mc * 36, width=36),
                    op0=OP.add, op1=OP.mult)

            # ---- 7. kse phases (fp32) + sins (bf16 out)
            kse = []
            for ax, wn in [(0, "fk5y"), (1, "fk5x")]:
                phK = psum.tile([128, 288], F32, space="PSUM", tag="psA",
                                bufs=3, name=f"phK{ax}")
                nc.tensor.matmul(out=phK[:], lhsT=wgs(wn, rows=5),
                                 rhs=kseRhs, start=True, stop=True)
                st = pool.tile([128, 288], BF16, name=f"kse{ax}")
                sin_reduce(phK[:], 288, st[:], f"k{ax}")
                kse.append(st)
            if debug:
                nc.sync.dma_start(out=dbg["d_kse0"][:], in_=kse[0][:])
            # prefetch the exp table right after the last sin
            wt2 = pool.tile([1, 1], F32)
            nc.scalar.activation(out=wt2[:], in_=wt[:], func=AF.Exp)

            # ---- 8. pos_k MLP (bf16)
            midK = []
            for mc in range(2):
                p = psum.tile([128, 288], F32, space="PSUM", tag="psA", bufs=3,
                              name=f"mkP{mc}")
                for kc in range(2):
                    nc.tensor.matmul(
                        out=p[:], lhsT=wfs("wk1", off=(kc * 2 + mc) * 128,
                                           width=128),
                        rhs=kse[kc][:], start=(kc == 0), stop=(kc == 1))
                t = pool.tile([128, 288], BF16, name=f"midK{mc}")
                nc.vector.tensor_scalar(out=t[:], in0=p[:],
                                        scalar1=wgs("bk1", off=mc, width=1),
                                        scalar2=0.0, op0=OP.add, op1=OP.max)
                midK.append(t)
            pkS = []
            for mc in range(2):
                p = psum.tile([128, 288], F32, space="PSUM", tag="psA", bufs=3,
                              name=f"pkP{mc}")
                for kc in range(2):
                    nc.tensor.matmul(
                        out=p[:], lhsT=wfs("wk2", off=(kc * 2 + mc) * 128,
                                           width=128),
                        rhs=midK[kc][:], start=(kc == 0), stop=(kc == 1))
                t = pool.tile([128, 288], BF16, name=f"pkS{mc}")
                nc.vector.tensor_scalar(out=t[:], in0=p[:],
                                        scalar1=wgs("bk2", off=mc, width=1),
                                        scalar2=None, op0=OP.add)
                pkS.append(t)
            if debug:
                nc.sync.dma_start(out=dbg["d_posk0"][:], in_=pkS[0][:])

            # ---- 9. bilinear combine: 4 fused ops per chunk on Pool.
            # gather quarters [c00|c01|c10|c11]; quarter j uses w4 col
            # [0, 2, 1, 3][j].
            sam = []
            for c, (c0, cn) in enumerate(CHUNKS):
                g = gA[c]
                t1 = pool.tile([128, 256], BF16, name=f"bt{c}")
                sm = pool.tile([128, 256], BF16, name=f"sam{c}")
                nc.vector.tensor_scalar(out=t1[:cn, :], in0=g[:cn, 0:256],
                                        scalar1=w4[c][:cn, 0:1], scalar2=None,
                                        op0=OP.mult)
                nc.vector.scalar_tensor_tensor(
                    out=t1[:cn, :], in0=g[:cn, 256:512],
                    scalar=w4[c][:cn, 2:3], in1=t1[:cn, :],
                    op0=OP.mult, op1=OP.add)
                nc.vector.scalar_tensor_tensor(
                    out=t1[:cn, :], in0=g[:cn, 512:768],
                    scalar=w4[c][:cn, 1:2], in1=t1[:cn, :],
                    op0=OP.mult, op1=OP.add)
                nc.vector.scalar_tensor_tensor(
                    out=sm[:cn, :], in0=g[:cn, 768:1024],
                    scalar=w4[c][:cn, 3:4], in1=t1[:cn, :],
                    op0=OP.mult, op1=OP.add)
                sam.append(sm)
            if debug:
                nc.sync.dma_start(out=dbg["d_sam0"][:], in_=sam[0][:])

            # ---- 10. transpose sampled to feature-major: 3 chunk transposes
            # per feature half into one psum bank, one copy out.
            samT = []
            for fc in range(2):
                tpB = psum.tile([128, 288], BF16, space="PSUM", tag="psA",
                                bufs=3, name=f"samTP{fc}")
                for c, (c0, cn) in enumerate(CHUNKS):
                    nc.tensor.transpose(
                        out=tpB[:, c0:c0 + cn],
                        in_=sam[c][:cn, fc * 128:(fc + 1) * 128],
                        identity=wfs("identB", rows=cn, width=cn))
                t = pool.tile([128, 288], BF16, name=f"samT{fc}")
                nc.scalar.copy(out=t[:], in_=tpB[:])
                samT.append(t)

            # ---- 11. conv: con_k / v = sampled @ [W_con_k | W_v]  (bf16)
            convP = []
            for mc in range(4):
                p = psum.tile([128, 288], F32, space="PSUM", tag="convP",
                              bufs=4, name=f"convP{mc}")
                for kc in range(2):
                    nc.tensor.matmul(
                        out=p[:], lhsT=wfs("wcat", off=(kc * 4 + mc) * 128,
                                           width=128),
                        rhs=samT[kc][:], start=(kc == 0), stop=(kc == 1))
                convP.append(p)
            vS = []
            for fc in range(2):
                t = pool.tile([128, 288], BF16, name=f"vS{fc}")
                nc.scalar.copy(out=t[:], in_=convP[2 + fc][:])
                vS.append(t)
            if debug:
                t = pool.tile([128, 288], F32)
                nc.scalar.copy(out=t[:], in_=convP[0][:])
                nc.sync.dma_start(out=dbg["d_conv0"][:], in_=t[:])

            # ---- 12. sim = scaled per-head dots via selection matmuls (bf16)
            simP = psum.tile([8, 288], F32, space="PSUM", tag="psA", bufs=3,
                             name="simP")
            pairs = [(convP[0][:], cqS, 0, "s0"), (convP[1][:], cqS, 1, "s1"),
                     (pkS[0][:], pqS, 0, "s0"), (pkS[1][:], pqS, 1, "s1")]
            for i, (kap, qt, mc, sname) in enumerate(pairs):
                tmp = pool.tile([128, 288], BF16, name=f"tmp{i}")
                qap = qt[:, mc * 36:(mc + 1) * 36]
                ta = tmp[:]
                nc.vector.tensor_tensor(
                    out=view3(ta, [[36, 8], [1, 36]]),
                    in0=AP(kap.tensor, kap.offset, [kap.ap[0], [36, 8], [1, 36]]),
                    in1=AP(qap.tensor, qap.offset, [qap.ap[0], [0, 8], [1, 36]]),
                    op=OP.mult)
                nc.tensor.matmul(out=simP[:], lhsT=wfs(sname, width=8),
                                 rhs=tmp[:], start=(i == 0), stop=(i == 3))
            if debug:
                t = pool.tile([8, 288], F32)
                nc.vector.tensor_copy(out=t[:], in_=simP[:])
                nc.sync.dma_start(out=dbg["d_sim"][:], in_=t[:])

            # ---- 13. softmax over the 8 keys: exp -> sum -> normalize
            # before head-expansion.  (|sim| small: exp w/o max-sub is safe.)
            ex = pool.tile([8, 288], BF16, name="ex")
            nc.scalar.activation(out=ex[:], in_=simP[:], func=AF.Exp)
            smt = pool.tile([8, 36], F32, name="smt")
            nc.vector.reduce_sum(out=smt[:], in_=view3(ex[:], [[1, 36], [36, 8]]),
                                 axis=mybir.AxisListType.X)
            rct = pool.tile([8, 36], F32, name="rct")
            nc.vector.reciprocal(out=rct[:], in_=smt[:])
            exn = pool.tile([8, 288], BF16, name="exn")
            rca = rct[:]
            nc.gpsimd.tensor_tensor(
                out=view3(exn[:], [[1, 36], [36, 8]]),
                in0=view3(ex[:], [[1, 36], [36, 8]]),
                in1=AP(rca.tensor, rca.offset, [rca.ap[0], [1, 36], [0, 8]]),
                op=OP.mult)
            if debug:
                nc.sync.dma_start(out=dbg["d_at"][:], in_=exn[:])

            # ---- 14. attn-weighted values (normalized attn, no re-expand)
            avT = []
            for fc in range(2):
                ae = psum.tile([128, 288], F32, space="PSUM", tag="psA",
                               bufs=3, name=f"aeP{fc}")
                nc.tensor.matmul(out=ae[:], lhsT=wfs(f"e{fc}", rows=8,
                                                     width=128),
                                 rhs=exn[:], start=True, stop=True)
                pr = pool.tile([128, 288], BF16, name=f"pr{fc}")
                nc.vector.tensor_tensor(out=pr[:], in0=vS[fc][:], in1=ae[:],
                                        op=OP.mult)
                av = pool.tile([128, 36], BF16, name=f"avT{fc}")
                with nc.allow_low_precision(reason="bf16 attn output is well "
                                            "within the 2e-2 tolerance"):
                    nc.vector.reduce_sum(out=av[:],
                                         in_=view3(pr[:], [[1, 36], [36, 8]]),
                                         axis=mybir.AxisListType.X)
                avT.append(av)
            if debug:
                nc.sync.dma_start(out=dbg["d_av0"][:], in_=avT[0][:])

            # ---- 15. out = attn_out @ W_out + b_out + identity
            oP = psum.tile([128, 288], F32, space="PSUM", tag="psA", bufs=3,
                           name="oP")
            for mc in range(2):
                for kc in range(2):
                    nc.tensor.matmul(
                        out=oP[:, mc * 36:(mc + 1) * 36],
                        lhsT=wfs("wout", off=(kc * 2 + mc) * 128, width=128),
                        rhs=avT[kc][:], start=(kc == 0), stop=(kc == 1))
            oT = pool.tile([128, 72], F32, name="oT")
            for mc in range(2):
                nc.scalar.activation(out=oT[:, mc * 36:(mc + 1) * 36],
                                     in_=oP[:, mc * 36:(mc + 1) * 36],
                                     func=AF.Identity,
                                     bias=wgs("bout", off=mc, width=1))
                nc.vector.tensor_tensor(out=oT[:, mc * 36:(mc + 1) * 36],
                                        in0=oT[:, mc * 36:(mc + 1) * 36],
                                        in1=xbs("deT", off=mc * 36, width=36),
                                        op=OP.add)
            ota = oT[:]
            oda = out[:]
            nc.sync.dma_start(
                out=AP(oda.tensor, oda.offset, [[36, 128], [128 * 36, 2], [1, 36]]),
                in_=AP(ota.tensor, ota.offset, [[72, 128], [36, 2], [1, 36]]))

    return nc


# ------------------------------------------------------------------- driver

def make_in_maps(dec_embed, bev_feat, query_scale, ref_points, weights):
    wf = pack_wf(weights)
    wg = pack_wg(weights)
    bevs = []
    for b in range(B):
        hwc = bev_feat[b].transpose(1, 2, 0).reshape(H * W, 256)
        bev_hwc = np.zeros((H * W, 512), np.float32)
        bev_hwc[:, 0:256] = hwc
        bev_hwc[:(H - 1) * W, 256:512] = hwc[W:]
        bevs.append(np.ascontiguousarray(bev_hwc.astype(NPBF)))
    in_maps = []
    for c in range(8):
        b, kh = c // 2, c % 2
        in_maps.append({
            "bev": bevs[b], "wf": wf, "wg": wg,
            "xbl": pack_xb(dec_embed, query_scale, ref_points, b, 3 * kh),
            "xh": pack_xh(dec_embed, b, 3 * kh),
        })
    return in_maps


def assemble_output(results):
    out = np.zeros((K, B, T, DIM), np.float32)
    for c in range(8):
        b, kh = c // 2, c % 2
        oc = results[c]["out"]                     # (256, 36)
        out[3 * kh:3 * kh + 3, b] = oc.T.reshape(3, T, DIM)
    return out


_WNAMES = ["W_con_q", "b_con_q", "W_con_k", "W_v", "Wq1", "bq1", "Wq2", "bq2",
           "Wk1", "bk1", "Wk2", "bk2", "Wo1", "bo1", "Wo2", "bo2",
           "W_out", "b_out"]


def kernel(**inputs):
    from concourse.bass_utils import run_bass_kernel_spmd
    dec_embed = np.asarray(inputs["dec_embed"], np.float32)
    bev_feat = np.asarray(inputs["bev_feat"], np.float32)
    query_scale = np.asarray(inputs["query_scale"], np.float32)
    ref_points = np.asarray(inputs["ref_points"], np.float32)
    weights = {n: np.asarray(inputs[n], np.float32) for n in _WNAMES}

    nc = build_nc(sim_mode=False, debug=False)
    split_multiwaits(nc)
    in_maps = make_in_maps(dec_embed, bev_feat, query_scale, ref_points, weights)
    res = run_bass_kernel_spmd(nc, in_maps, list(range(8)))
    return assemble_output(res.results)


# revision 10
# speedup vs baseline: 1.5778x; 1.1794x over previous
"""BEV deformable cross-attention kernel for 8 Trainium2 NeuronCores.

Strategy (per core): data-parallel over (B x K-half): core c handles batch
b = c//2 and modes k in {3*(c%2) .. +3}, i.e. 36 queries, 288 sample points.

Key algebraic move: grid_sample(conv1x1(bev)) == conv1x1(grid_sample(bev)),
so instead of materializing the two full (256,200,200) conv maps we gather
only the 4 bilinear corners of the 288 sample points from a host-transposed
HWC copy of bev_feat (channels contiguous per pixel -> 2KB indirect reads),
interpolate in 256-d, then apply the 1x1 convs to 288 vectors.

v2 perf notes vs the first working version:
- all fat matmuls run in bf16 (fp32 matmuls lower to 2 half-rate HW passes);
  only the sine-phase matmuls and the geometry path stay fp32.
- weights/inputs land via parallel DMA queues (gpsimd + sync) so the first
  matmul no longer waits on unrelated blobs.
- gelu is computed through the Silu table (gelu(x) ~ x*sigmoid(1.702x),
  exact for the tiny pre-activations here) so {silu,tanh,sin} share one
  activation table and only one mid-kernel table switch (exp) remains.
- the bilinear combine is 4 fused scalar_tensor_tensor ops per chunk on the
  Pool engine; softmax normalizes before head-expansion (no re-expand mm).
"""
import numpy as np
import ml_dtypes

import concourse.bass as bass
import concourse.mybir as mybir
import concourse.tile as tile_mod
from concourse.bass import AP, IndirectOffsetOnAxis

F32 = mybir.dt.float32
BF16 = mybir.dt.bfloat16
I32 = mybir.dt.int32
AF = mybir.ActivationFunctionType
OP = mybir.AluOpType
NPBF = ml_dtypes.bfloat16

# problem constants (hardcoded per contract)
K, B, T, DIM = 6, 4, 12, 256
H, W = 200, 200
HALF = 256
G = 8                      # offset groups == sample points per query
NQ = 3 * T                 # queries per core = 36
NPT = NQ * G               # points per core = 288
OFFSET_SCALE = 4.0
PIX_SCALE = float(W / 102.4)          # 1.953125
PIX_BIAS = float(W / 2.0 - 0.5)       # 99.5
SCALE = 64 ** -0.5                    # 0.125
TWO_PI = float(2 * np.pi)
RC = float(3 * 2 ** 22)               # 1.5*2^23 rint magic constant
SILU_A = 1.702                        # gelu(x) ~ silu(1.702 x)/1.702
CHUNKS = [(0, 128), (128, 128), (256, 32)]   # point chunks (start, size)

# ---------------------------------------------------------------- blob layout


class Alloc:
    def __init__(self):
        self.pos = 0
        self.slices = {}

    def add(self, name, width):
        self.slices[name] = (self.pos, width)
        self.pos += width

    def __getitem__(self, name):
        return self.slices[name]


# bf16 matmul-weight blob; split points F0/F1/F2 are separate DMAs so the
# critical-path prefix (wconq) lands first.
WF_ITEMS = [("wconq", 512),                                        # F0
            ("bdh", 512), ("wo2t", 2), ("wo2b", 2),                # F1...
            ("wq1", 512), ("wq2", 512),
            ("s0", 8), ("s1", 8), ("e0", 128), ("e1", 128), ("identB", 128),
            ("wk1", 512), ("wk2", 512), ("wcat", 1024), ("wout", 512)]  # F2
F0_END = 512
F1_END = 512 + 512 + 4 + 1024 + 16 + 256 + 128     # 2452

# fp32 misc blob: biases (as (128,2) column pairs), geometry consts,
# sine-phase weights.
WG_ITEMS = [("bconq", 2), ("bo1s", 1), ("bo2", 1), ("sc4pm", 2),
            ("fq2", 128), ("fk5x", 128), ("fk5y", 128), ("id2", 2),
            ("bq1", 2), ("bq2", 2), ("bk1", 2), ("bk2", 2), ("bout", 2)]

# fp32 per-core input blob. rpo holds [tanh_x; tanh_y; rpx; rpy; ones] rows:
# partitions 0:2 are blank (filled by the on-device tanh), 2:5 host data.
XB_ITEMS = [("deT", 72), ("qsT", 72), ("rpyx1", 72), ("rpo", 288), ("bpm", 6)]


def _layout(items):
    a = Alloc()
    for nm, wd in items:
        a.add(nm, wd)
    return a


WF_LAY = _layout(WF_ITEMS)
WG_LAY = _layout(WG_ITEMS)
XB_LAY = _layout(XB_ITEMS)


def _put_mm(dst, lay, name, w256):
    """(256, Mout) -> (kc, mc) blocks of (128, 128) at s + (kc*mcs+mc)*128."""
    s, _ = lay[name]
    mcs = w256.shape[1] // 128
    for kc in range(2):
        for mc in range(mcs):
            blk = w256[kc * 128:(kc + 1) * 128, mc * 128:(mc + 1) * 128]
            off = (kc * mcs + mc) * 128
            dst[:, s + off: s + off + 128] = blk


def pack_wf(weights):
    wf = np.zeros((128, WF_LAY.pos), np.float32)
    lay = WF_LAY

    def put(name, arr, rows=128):
        s, _ = lay[name]
        wf[:rows, s: s + arr.shape[1]] = arr

    _put_mm(wf, lay, "wconq", weights["W_con_q"])
    # block-diag Wo1: block j covers groups (2j, 2j+1); even j from feature
    # chunk 0 rows, odd j from chunk 1 rows.
    s, _ = lay["bdh"]
    wo1 = weights["Wo1"]  # (32, 64)
    for j in range(4):
        blk = np.zeros((128, 128), np.float32)
        if j % 2 == 0:
            blk[0:32, 0:64] = wo1
            blk[32:64, 64:128] = wo1
        else:
            blk[64:96, 0:64] = wo1
            blk[96:128, 64:128] = wo1
        wf[:, s + j * 128: s + (j + 1) * 128] = blk
    wo2 = weights["Wo2"] / SILU_A          # undo the silu input scale
    top = np.zeros((128, 2), np.float32); top[0:64] = wo2
    bot = np.zeros((128, 2), np.float32); bot[64:128] = wo2
    put("wo2t", top); put("wo2b", bot)
    _put_mm(wf, lay, "wq1", weights["Wq1"])
    _put_mm(wf, lay, "wq2", weights["Wq2"])
    _put_mm(wf, lay, "wk1", weights["Wk1"])
    _put_mm(wf, lay, "wk2", weights["Wk2"])
    wcat = np.concatenate([weights["W_con_k"], weights["W_v"]], axis=1)
    _put_mm(wf, lay, "wcat", wcat)
    _put_mm(wf, lay, "wout", weights["W_out"])
    d = np.arange(128)
    s0 = np.zeros((128, 8), np.float32); s0[d, d // 32] = SCALE
    s1 = np.zeros((128, 8), np.float32); s1[d, 4 + d // 32] = SCALE
    put("s0", s0); put("s1", s1)
    e0 = np.zeros((8, 128), np.float32); e0[d // 32, d] = 1.0
    e1 = np.zeros((8, 128), np.float32); e1[4 + d // 32, d] = 1.0
    put("e0", e0, rows=8); put("e1", e1, rows=8)
    put("identB", np.eye(128, dtype=np.float32))
    return wf.astype(NPBF)


def _freq_shift():
    i64 = np.arange(128) // 2
    freq = (TWO_PI / (10000.0 ** (i64 / 64.0))).astype(np.float32)
    shift = np.where(np.arange(128) % 2 == 1, np.pi / 2, 0.0).astype(np.float32)
    return freq, shift


def pack_wg(weights):
    wg = np.zeros((128, WG_LAY.pos), np.float32)
    lay = WG_LAY

    def put(name, arr, rows=128):
        s, _ = lay[name]
        wg[:rows, s: s + arr.shape[1]] = arr

    put("bconq", weights["b_con_q"].reshape(2, 128).T)
    put("bo1s", SILU_A * np.tile(weights["bo1"], 2)[:, None])
    put("bo2", weights["bo2"][:, None], rows=2)
    put("sc4pm", np.tile(np.array([[4 * PIX_SCALE, -4 * PIX_SCALE]],
                                  np.float32), (128, 1)))
    freq, shift = _freq_shift()
    put("fq2", np.stack([freq, shift]), rows=2)
    fk5x = np.zeros((5, 128), np.float32)
    fk5x[0] = 4 * freq; fk5x[2] = freq; fk5x[4] = shift
    fk5y = np.zeros((5, 128), np.float32)
    fk5y[1] = 4 * freq; fk5y[3] = freq; fk5y[4] = shift
    put("fk5x", fk5x, rows=5)
    put("fk5y", fk5y, rows=5)
    put("id2", np.eye(2, dtype=np.float32), rows=2)
    put("bq1", weights["bq1"].reshape(2, 128).T)
    put("bq2", weights["bq2"].reshape(2, 128).T)
    put("bk1", weights["bk1"].reshape(2, 128).T)
    put("bk2", weights["bk2"].reshape(2, 128).T)
    put("bout", weights["b_out"].reshape(2, 128).T)
    return wg


def pack_xb(dec_embed, query_scale, ref_points, b, k0):
    lay = XB_LAY
    xb = np.zeros((128, lay.pos), np.float32)
    de = dec_embed[k0:k0 + 3, b].reshape(NQ, DIM)       # (36, 256)
    qs = query_scale[k0:k0 + 3, b].reshape(NQ, DIM)
    rp = ref_points[k0:k0 + 3, b].reshape(NQ, 2)

    s, _ = lay["deT"]
    xb[:, s: s + 36] = de.T[:128]
    xb[:, s + 36: s + 72] = de.T[128:]
    s, _ = lay["qsT"]
    xb[:, s: s + 36] = qs.T[:128]
    xb[:, s + 36: s + 72] = qs.T[128:]
    s, _ = lay["rpyx1"]
    xb[0, s: s + 36] = rp[:, 1]                         # y first (DAB order)
    xb[0, s + 36: s + 72] = rp[:, 0]
    xb[1, s: s + 72] = 1.0
    s, _ = lay["rpo"]
    rpe = np.tile(rp.T, (1, 8))                         # g-major: col = g*36+q
    xb[2, s: s + 288] = rpe[0]
    xb[3, s: s + 288] = rpe[1]
    xb[4, s: s + 288] = 1.0
    s, _ = lay["bpm"]
    bx = PIX_SCALE * rpe[0] + PIX_BIAS
    by = -PIX_SCALE * rpe[1] + PIX_BIAS
    for c, (c0, cn) in enumerate(CHUNKS):
        xb[:cn, s + 2 * c] = bx[c0:c0 + cn]
        xb[:cn, s + 2 * c + 1] = by[c0:c0 + cn]
    return xb


def pack_xh(dec_embed, b, k0):
    de = dec_embed[k0:k0 + 3, b].reshape(NQ, DIM)
    xh = np.zeros((128, 72), np.float32)
    xh[:, 0:36] = de.T[:128]
    xh[:, 36:72] = de.T[128:]
    return xh.astype(NPBF)


# --------------------------------------------------------------- tile patches

def _split_drain_and_barrier(self, tick_clock, wait_clock):
    nc = self.nc
    drain_inst = nc.sync.drain()
    wait_clock.add_sem_waits(
        drain_inst.ins, tile_mod.ScopedClock({None: tick_clock.global_clock})
    )
    si = drain_inst.ins.sync_info
    waits = list(si.on_wait)
    if len(waits) > 1:
        si.on_wait = waits[:1]
        for i in range(1, len(waits)):
            extra = nc.sync.drain()
            extra.ins.sync_info = type(si)(on_wait=waits[i: i + 1], on_update=[])
    nc.all_engine_barrier()
    assert self.sems is not None
    popped = nc._tile_sem_poison_stack.pop()
    assert popped is self._sem_poison
    nc.clear_and_free_semaphores(list(self.sems.allocated().values()))


def split_multiwaits(nc):
    """walrus codegen supports a single sync-wait per instruction; split."""
    f = nc.m.functions[0]
    for blk in f.blocks:
        todo = [i for i in blk.instructions
                if i.sync_info is not None and len(i.sync_info.on_wait) > 1]
        for inst in todo:
            si = inst.sync_info
            waits = list(si.on_wait)
            nops = []
            for w in waits[:-1]:
                bi = nc.engines[inst.engine].nop(nofuse=True)
                ni = bi.ins
                for b2 in f.blocks:
                    if b2.instructions and b2.instructions[-1] is ni:
                        b2.instructions.pop()
                        break
                ni.sync_info = type(si)(on_wait=[w], on_update=[])
                nops.append(ni)
            si.on_wait = [waits[-1]]
            pos = blk.instructions.index(inst)
            blk.instructions[pos:pos] = nops


_PATCHED = False


def patch_tile():
    global _PATCHED
    if not _PATCHED:
        tile_mod.TileContext._drain_and_barrier = _split_drain_and_barrier
        _PATCHED = True


# ---------------------------------------------------------------- the kernel

def view3(ap, dims):
    """3D AP view over a 2D tile AP: dims = [[step,count],...] after ap[0]."""
    return AP(ap.tensor, ap.offset, [ap.ap[0]] + dims)


def build_nc(sim_mode=False, debug=False):
    patch_tile()
    nc = bass.Bass("TRN2")

    # row-pair interleaved bf16: bev[y*W+x] = [feat(y,x) | feat(y+1,x)]
    bev = nc.dram_tensor("bev", [H * W, 512], BF16, kind="ExternalInput")
    wfD = nc.dram_tensor("wf", [128, WF_LAY.pos], BF16, kind="ExternalInput")
    wgD = nc.dram_tensor("wg", [128, WG_LAY.pos], F32, kind="ExternalInput")
    xbD = nc.dram_tensor("xbl", [128, XB_LAY.pos], F32, kind="ExternalInput")
    xhD = nc.dram_tensor("xh", [128, 72], BF16, kind="ExternalInput")
    out = nc.dram_tensor("out", [256, NQ], F32, kind="ExternalOutput")

    dbg = {}
    if debug:
        for nm, shp, dt in [
            ("d_pix", [128, 2], F32), ("d_idx", [128, 1], I32),
            ("d_w40", [128, 4], F32), ("d_sam0", [128, 256], BF16),
            ("d_cq0", [128, 36], BF16), ("d_h", [128, 144], BF16),
            ("d_qse0", [128, 36], BF16), ("d_kse0", [128, 288], BF16),
            ("d_posk0", [128, 288], BF16), ("d_conv0", [128, 288], F32),
            ("d_sim", [8, 288], F32), ("d_at", [8, 288], BF16),
            ("d_av0", [128, 36], BF16),
        ]:
            dbg[nm] = nc.dram_tensor(nm, shp, dt, kind="ExternalOutput")

    with tile_mod.TileContext(nc) as tc:
        with (
            tc.tile_pool(name="sbuf", bufs=1) as pool,
            tc.tile_pool(name="psum", bufs=1, space="PSUM") as psum,
        ):
            # warm the {silu,tanh,sin} table during the input DMAs
            wt = pool.tile([1, 1], F32)
            nc.vector.memset(wt[:], 0.0)
            warm = pool.tile([1, 1], F32)
            nc.scalar.activation(out=warm[:], in_=wt[:],
                                 func=AF.Sigmoid if sim_mode else AF.Silu,
                                 bias=0.0)

            # ---- input DMAs.  wf is three separate tiles so the con_q
            # matmul only waits on the wconq prefix, not the whole blob.
            xh = pool.tile([128, 72], BF16)
            nc.gpsimd.dma_start(out=xh[:], in_=xhD[:])
            wf0 = pool.tile([128, F0_END], BF16)
            nc.gpsimd.dma_start(out=wf0[:], in_=wfD[:, 0:F0_END])
            wg = pool.tile([128, WG_LAY.pos], F32)
            nc.sync.dma_start(out=wg[:], in_=wgD[:])
            xb = pool.tile([128, XB_LAY.pos], F32)
            nc.sync.dma_start(out=xb[:], in_=xbD[:])
            wf1 = pool.tile([128, F1_END - F0_END], BF16)
            nc.gpsimd.dma_start(out=wf1[:], in_=wfD[:, F0_END:F1_END])
            wf2 = pool.tile([128, WF_LAY.pos - F1_END], BF16)
            nc.gpsimd.dma_start(out=wf2[:], in_=wfD[:, F1_END:WF_LAY.pos])

            def wfs(name, rows=128, off=0, width=None):
                s, wd = WF_LAY[name]
                if width is None:
                    width = wd - off
                if s < F0_END:
                    t, base = wf0, 0
                elif s < F1_END:
                    t, base = wf1, F0_END
                else:
                    t, base = wf2, F1_END
                return t[0:rows, s - base + off: s - base + off + width]

            def wgs(name, rows=128, off=0, width=None):
                s, wd = WG_LAY[name]
                if width is None:
                    width = wd - off
                return wg[0:rows, s + off: s + off + width]

            def xbs(name, rows=128, off=0, width=None):
                s, wd = XB_LAY[name]
                if width is None:
                    width = wd - off
                return xb[0:rows, s + off: s + off + width]

            # ---- 1. con_q = de @ W_con_q + b   (bf16)
            cqP = psum.tile([128, 288], F32, space="PSUM", tag="psA", bufs=3,
                            name="cqP")
            for mc in range(2):
                for kc in range(2):
                    nc.tensor.matmul(
                        out=cqP[:, mc * 36:(mc + 1) * 36],
                        lhsT=wfs("wconq", off=(kc * 2 + mc) * 128, width=128),
                        rhs=xh[:, kc * 36:(kc + 1) * 36],
                        start=(kc == 0), stop=(kc == 1))
            cqS = pool.tile([128, 72], BF16, name="cqS")
            for mc in range(2):
                nc.scalar.activation(out=cqS[:, mc * 36:(mc + 1) * 36],
                                     in_=cqP[:, mc * 36:(mc + 1) * 36],
                                     func=AF.Identity,
                                     bias=wgs("bconq", off=mc, width=1))
            if debug:
                nc.sync.dma_start(out=dbg["d_cq0"][:], in_=cqS[:, 0:36])

            # ---- 2. h = gelu(grouped con_q @ Wo1 + bo1) via silu table
            hP = psum.tile([128, 288], F32, space="PSUM", tag="psA", bufs=3,
                           name="hP")
            for j in range(4):
                cc = j // 2
                nc.tensor.matmul(
                    out=hP[:, j * 36:(j + 1) * 36],
                    lhsT=wfs("bdh", off=j * 128, width=128),
                    rhs=cqS[:, cc * 36:(cc + 1) * 36], start=True, stop=True)
            hS = pool.tile([128, 144], BF16, name="hS")
            if sim_mode:
                hx = pool.tile([128, 144], F32)
                nc.scalar.activation(out=hx[:], in_=hP[:, :144],
                                     func=AF.Identity, scale=SILU_A,
                                     bias=wgs("bo1s"))
                he = pool.tile([128, 144], F32)
                nc.scalar.activation(out=he[:], in_=hx[:], func=AF.Sigmoid,
                                     bias=0.0)
                nc.vector.tensor_tensor(out=hS[:], in0=hx[:], in1=he[:],
                                        op=OP.mult)
            else:
                nc.scalar.activation(out=hS[:], in_=hP[:, :144], func=AF.Silu,
                                     scale=SILU_A, bias=wgs("bo1s"))
            if debug:
                nc.sync.dma_start(out=dbg["d_h"][:], in_=hS[:])

            # ---- 3. offsets -> tanh into xb rows 0:2 of the rpo region
            offP = psum.tile([2, 288], F32, space="PSUM", tag="psA", bufs=3,
                             name="offP")
            for m, wn in [(0, "wo2t"), (1, "wo2b")]:
                nc.tensor.matmul(
                    out=offP[:, m * 144:(m + 1) * 144],
                    lhsT=wfs(wn, width=2), rhs=hS[:], start=True, stop=True)
            # qse phase matmul slotted here: fills the PE bubble while the
            # tanh runs on ACT.
            phQ = psum.tile([128, 288], F32, space="PSUM", tag="psA", bufs=3,
                            name="phQ")
            nc.tensor.matmul(out=phQ[:, :72], lhsT=wgs("fq2", rows=2),
                             rhs=xbs("rpyx1", rows=2), start=True, stop=True)
            s_rpo, _ = XB_LAY["rpo"]
            kra = xb[0:2, s_rpo:s_rpo + 288]
            opa = offP[:]
            nc.scalar.activation(
                out=AP(kra.tensor, kra.offset,
                       [kra.ap[0], [72, 4], [36, 2], [1, 36]]),
                in_=AP(opa.tensor, opa.offset,
                       [opa.ap[0], [36, 4], [144, 2], [1, 36]]),
                func=AF.Tanh, bias=wgs("bo2", rows=2, width=1))
            kseRhs = xb[0:5, s_rpo:s_rpo + 288]

            # ---- 4. per-chunk geometry -> indices -> gathers (bf16 rows)
            s_bpm, _ = XB_LAY["bpm"]
            frs, idxI, gA, w4 = [], [], [], []
            pix0 = None
            for c, (c0, cn) in enumerate(CHUNKS):
                tp = psum.tile([128, 2], F32, space="PSUM", tag="psA", bufs=3,
                               name=f"tpP{c}")
                nc.tensor.transpose(out=tp[:cn, :], in_=kseRhs[0:2, c0:c0 + cn],
                                    identity=wgs("id2", rows=2, width=2))
                pix = pool.tile([128, 2], F32, name=f"pix{c}")
                if c == 0:
                    pix0 = pix
                nc.vector.tensor_tensor(out=pix[:cn, :], in0=tp[:cn, :],
                                        in1=wgs("sc4pm", rows=cn, width=2),
                                        op=OP.mult)
                nc.vector.tensor_tensor(
                    out=pix[:cn, :], in0=pix[:cn, :],
                    in1=xb[0:cn, s_bpm + 2 * c: s_bpm + 2 * c + 2], op=OP.add)
                f0 = pool.tile([128, 2], F32, name=f"f0{c}")
                nc.vector.tensor_scalar(out=f0[:cn, :], in0=pix[:cn, :],
                                        scalar1=-0.5, scalar2=float(RC),
                                        op0=OP.add, op1=OP.add)
                nc.vector.tensor_scalar(out=f0[:cn, :], in0=f0[:cn, :],
                                        scalar1=float(-RC), scalar2=None,
                                        op0=OP.add)
                fr = pool.tile([128, 2], F32, name=f"fr{c}")
                nc.vector.tensor_tensor(out=fr[:cn, :], in0=pix[:cn, :],
                                        in1=f0[:cn, :], op=OP.subtract)
                frs.append(fr)
                idf = pool.tile([128, 1], F32, name=f"idf{c}")
                nc.vector.scalar_tensor_tensor(
                    out=idf[:cn, :], in0=f0[:cn, 1:2], scalar=float(W),
                    in1=f0[:cn, 0:1], op0=OP.mult, op1=OP.add)
                ii = pool.tile([128, 1], I32, name=f"idxI{c}")
                nc.vector.tensor_copy(out=ii[:cn, :], in_=idf[:cn, :])
                idxI.append(ii)
                ga = pool.tile([128, 1024], BF16, name=f"gA{c}")
                nc.gpsimd.indirect_dma_start(
                    out=ga[:cn, :], out_offset=None, in_=bev[:],
                    in_offset=IndirectOffsetOnAxis(ap=ii[:cn, :], axis=0))
                gA.append(ga)
            # bilinear weights (Pc, 4) = [w00, w10, w01, w11]
            for c, (c0, cn) in enumerate(CHUNKS):
                fr = frs[c]
                wxp = pool.tile([128, 2], F32, name=f"wxp{c}")
                nc.vector.tensor_scalar(out=wxp[:cn, 0:1], in0=fr[:cn, 0:1],
                                        scalar1=-1.0, scalar2=1.0,
                                        op0=OP.mult, op1=OP.add)
                nc.vector.tensor_copy(out=wxp[:cn, 1:2], in_=fr[:cn, 0:1])
                wyp = pool.tile([128, 2], F32, name=f"wyp{c}")
                nc.vector.tensor_scalar(out=wyp[:cn, 0:1], in0=fr[:cn, 1:2],
                                        scalar1=-1.0, scalar2=1.0,
                                        op0=OP.mult, op1=OP.add)
                nc.vector.tensor_copy(out=wyp[:cn, 1:2], in_=fr[:cn, 1:2])
                w4c = pool.tile([128, 4], F32, name=f"w4{c}")
                wxa = wxp[:cn, :]
                wya = wyp[:cn, :]
                nc.vector.tensor_tensor(
                    out=view3(w4c[:cn, :], [[2, 2], [1, 2]]),
                    in0=AP(wxa.tensor, wxa.offset, [wxa.ap[0], [0, 2], [1, 2]]),
                    in1=AP(wya.tensor, wya.offset, [wya.ap[0], [1, 2], [0, 2]]),
                    op=OP.mult)
                w4.append(w4c)
            if debug:
                nc.sync.dma_start(out=dbg["d_pix"][:], in_=pix0[:])
                nc.sync.dma_start(out=dbg["d_idx"][:], in_=idxI[0][:])
                nc.sync.dma_start(out=dbg["d_w40"][:], in_=w4[0][:])

            # ---- 5. kse phase matmuls (fp32) right after the tp transposes:
            # they only need the tanh, and fill the PE gather window.
            phK = []
            for ax, wn in [(0, "fk5y"), (1, "fk5x")]:
                p = psum.tile([128, 288], F32, space="PSUM", tag="psA",
                              bufs=3, name=f"phK{ax}")
                nc.tensor.matmul(out=p[:], lhsT=wgs(wn, rows=5),
                                 rhs=kseRhs, start=True, stop=True)
                phK.append(p)

            # ---- 6. qse sin (range reduce on DVE, m1 inline)
            qse = pool.tile([128, 72], BF16, name="qse")
            m1q = pool.tile([128, 72], F32, name="m1q")
            nc.vector.tensor_scalar(out=m1q[:], in0=phQ[:, :72],
                                    scalar1=float(1.0 / TWO_PI),
                                    scalar2=RC, op0=OP.mult, op1=OP.add)
            nc.vector.tensor_scalar(out=m1q[:], in0=m1q[:], scalar1=-RC,
                                    scalar2=-TWO_PI, op0=OP.add, op1=OP.mult)
            ytq = pool.tile([128, 72], F32, name="ytq")
            nc.vector.tensor_tensor(out=ytq[:], in0=phQ[:, :72], in1=m1q[:],
                                    op=OP.add)
            nc.vector.tensor_scalar(out=ytq[:], in0=ytq[:],
                                    scalar1=float(np.pi),
                                    scalar2=float(-np.pi),
                                    op0=OP.min, op1=OP.max)
            nc.scalar.activation(out=qse[:], in_=ytq[:], func=AF.Sin)
            if debug:
                nc.sync.dma_start(out=dbg["d_qse0"][:], in_=qse[:, 0:36])

            # ---- 7. pos_q MLP (bf16, overlaps the gathers)
            mqP = psum.tile([128, 288], F32, space="PSUM", tag="psA", bufs=3,
                            name="mqP")
            for mc in range(2):
                for kc in range(2):
                    nc.tensor.matmul(
                        out=mqP[:, mc * 36:(mc + 1) * 36],
                        lhsT=wfs("wq1", off=(kc * 2 + mc) * 128, width=128),
                        rhs=qse[:, kc * 36:(kc + 1) * 36],
                        start=(kc == 0), stop=(kc == 1))
            midQ = pool.tile([128, 72], BF16, name="midQ")
            for mc in range(2):
                nc.vector.tensor_scalar(
                    out=midQ[:, mc * 36:(mc + 1) * 36],
                    in0=mqP[:, mc * 36:(mc + 1) * 36],
                    scalar1=wgs("bq1", off=mc, width=1), scalar2=0.0,
                    op0=OP.add, op1=OP.max)
            pqP = psum.tile([128, 288], F32, space="PSUM", tag="psA", bufs=3,
                            name="pqP")
            for mc in range(2):
                for kc in range(2):
                    nc.tensor.matmul(
                        out=pqP[:, mc * 36:(mc + 1) * 36],
                        lhsT=wfs("wq2", off=(kc * 2 + mc) * 128, width=128),
                        rhs=midQ[:, kc * 36:(kc + 1) * 36],
                        start=(kc == 0), stop=(kc == 1))
            pqS = pool.tile([128, 72], BF16, name="pqS")
            for mc in range(2):
                nc.vector.scalar_tensor_tensor(
                    out=pqS[:, mc * 36:(mc + 1) * 36],
                    in0=pqP[:, mc * 36:(mc + 1) * 36],
                    scalar=wgs("bq2", off=mc, width=1),
                    in1=xbs("qsT", off=mc * 36, width=36),
                    op0=OP.add, op1=OP.mult)

            # ---- 8. kse sins: m1 on ACT (idle there), k2/y/clip on DVE
            kse = []
            for ax in range(2):
                m1 = pool.tile([128, 288], F32, name=f"m1k{ax}")
                nc.scalar.activation(out=m1[:], in_=phK[ax][:], func=AF.Copy,
                                     scale=float(1.0 / TWO_PI), bias=float(RC))
                nc.vector.tensor_scalar(out=m1[:], in0=m1[:], scalar1=-RC,
                                        scalar2=-TWO_PI, op0=OP.add,
                                        op1=OP.mult)
                yt = pool.tile([128, 288], F32, name=f"ytk{ax}")
                nc.vector.tensor_tensor(out=yt[:], in0=phK[ax][:], in1=m1[:],
                                        op=OP.add)
                nc.vector.tensor_scalar(out=yt[:], in0=yt[:],
                                        scalar1=float(np.pi),
                                        scalar2=float(-np.pi),
                                        op0=OP.min, op1=OP.max)
                st = pool.tile([128, 288], BF16, name=f"kse{ax}")
                nc.scalar.activation(out=st[:], in_=yt[:], func=AF.Sin)
                kse.append(st)
            if debug:
                nc.sync.dma_start(out=dbg["d_kse0"][:], in_=kse[0][:])
            # exp-table prefetch; reading kse[1] pins it after the last sin
            wt2 = pool.tile([1, 1], F32)
            nc.scalar.activation(out=wt2[:], in_=kse[1][0:1, 0:1], func=AF.Exp)

            # ---- 9. bilinear combine: 4 fused ops per chunk on DVE.
            # gather quarters [c00|c01|c10|c11]; quarter j uses w4 col
            # [0, 2, 1, 3][j].
            sam = []
            for c, (c0, cn) in enumerate(CHUNKS):
                g = gA[c]
                t1 = pool.tile([128, 256], BF16, name=f"bt{c}")
                sm = pool.tile([128, 256], BF16, name=f"sam{c}")
                nc.vector.tensor_scalar(out=t1[:cn, :], in0=g[:cn, 0:256],
                                        scalar1=w4[c][:cn, 0:1], scalar2=None,
                                        op0=OP.mult)
                nc.vector.scalar_tensor_tensor(
                    out=t1[:cn, :], in0=g[:cn, 256:512],
                    scalar=w4[c][:cn, 2:3], in1=t1[:cn, :],
                    op0=OP.mult, op1=OP.add)
                nc.vector.scalar_tensor_tensor(
                    out=t1[:cn, :], in0=g[:cn, 512:768],
                    scalar=w4[c][:cn, 1:2], in1=t1[:cn, :],
                    op0=OP.mult, op1=OP.add)
                nc.vector.scalar_tensor_tensor(
                    out=sm[:cn, :], in0=g[:cn, 768:1024],
                    scalar=w4[c][:cn, 3:4], in1=t1[:cn, :],
                    op0=OP.mult, op1=OP.add)
                sam.append(sm)
            if debug:
                nc.sync.dma_start(out=dbg["d_sam0"][:], in_=sam[0][:])

            # ---- 10+11. transposes to feature-major interleaved with the
            # pos_k MLP on PE (ordered by operand readiness).
            samTP = []
            for fc in range(2):
                samTP.append(psum.tile([128, 288], BF16, space="PSUM",
                                       tag="psA", bufs=3, name=f"samTP{fc}"))

            def transpose_chunk(c):
                c0, cn = CHUNKS[c]
                for fc in range(2):
                    nc.tensor.transpose(
                        out=samTP[fc][:, c0:c0 + cn],
                        in_=sam[c][:cn, fc * 128:(fc + 1) * 128],
                        identity=wfs("identB", rows=cn, width=cn))

            transpose_chunk(0)
            midK = []
            mkP = []
            for mc in range(2):
                p = psum.tile([128, 288], F32, space="PSUM", tag="psA", bufs=3,
                              name=f"mkP{mc}")
                for kc in range(2):
                    nc.tensor.matmul(
                        out=p[:], lhsT=wfs("wk1", off=(kc * 2 + mc) * 128,
                                           width=128),
                        rhs=kse[kc][:], start=(kc == 0), stop=(kc == 1))
                mkP.append(p)
                t = pool.tile([128, 288], BF16, name=f"midK{mc}")
                nc.scalar.activation(out=t[:], in_=p[:], func=AF.Relu,
                                     bias=wgs("bk1", off=mc, width=1))
                midK.append(t)
            transpose_chunk(1)
            pkS = []
            for mc in range(2):
                p = psum.tile([128, 288], F32, space="PSUM", tag="psA", bufs=3,
                              name=f"pkP{mc}")
                for kc in range(2):
                    nc.tensor.matmul(
                        out=p[:], lhsT=wfs("wk2", off=(kc * 2 + mc) * 128,
                                           width=128),
                        rhs=midK[kc][:], start=(kc == 0), stop=(kc == 1))
                t = pool.tile([128, 288], BF16, name=f"pkS{mc}")
                nc.scalar.activation(out=t[:], in_=p[:], func=AF.Identity,
                                     bias=wgs("bk2", off=mc, width=1))
                pkS.append(t)
            transpose_chunk(2)
            if debug:
                nc.sync.dma_start(out=dbg["d_posk0"][:], in_=pkS[0][:])
            samT = []
            for fc in range(2):
                t = pool.tile([128, 288], BF16, name=f"samT{fc}")
                nc.scalar.copy(out=t[:], in_=samTP[fc][:])
                samT.append(t)

            # ---- 11b. conv: con_k / v = sampled @ [W_con_k | W_v]  (bf16)
            convP = []
            for mc in range(4):
                p = psum.tile([128, 288], F32, space="PSUM", tag="convP",
                              bufs=4, name=f"convP{mc}")
                for kc in range(2):
                    nc.tensor.matmul(
                        out=p[:], lhsT=wfs("wcat", off=(kc * 4 + mc) * 128,
                                           width=128),
                        rhs=samT[kc][:], start=(kc == 0), stop=(kc == 1))
                convP.append(p)
            vS = []
            for fc in range(2):
                t = pool.tile([128, 288], BF16, name=f"vS{fc}")
                nc.scalar.copy(out=t[:], in_=convP[2 + fc][:])
                vS.append(t)
            if debug:
                t = pool.tile([128, 288], F32)
                nc.scalar.copy(out=t[:], in_=convP[0][:])
                nc.sync.dma_start(out=dbg["d_conv0"][:], in_=t[:])

            # ---- 12. sim = scaled per-head dots via selection matmuls (bf16)
            simP = psum.tile([8, 288], F32, space="PSUM", tag="psA", bufs=3,
                             name="simP")
            pairs = [(convP[0][:], cqS, 0, "s0"), (convP[1][:], cqS, 1, "s1"),
                     (pkS[0][:], pqS, 0, "s0"), (pkS[1][:], pqS, 1, "s1")]
            for i, (kap, qt, mc, sname) in enumerate(pairs):
                tmp = pool.tile([128, 288], BF16, name=f"tmp{i}")
                qap = qt[:, mc * 36:(mc + 1) * 36]
                ta = tmp[:]
                nc.vector.tensor_tensor(
                    out=view3(ta, [[36, 8], [1, 36]]),
                    in0=AP(kap.tensor, kap.offset, [kap.ap[0], [36, 8], [1, 36]]),
                    in1=AP(qap.tensor, qap.offset, [qap.ap[0], [0, 8], [1, 36]]),
                    op=OP.mult)
                nc.tensor.matmul(out=simP[:], lhsT=wfs(sname, width=8),
                                 rhs=tmp[:], start=(i == 0), stop=(i == 3))
            if debug:
                t = pool.tile([8, 288], F32)
                nc.vector.tensor_copy(out=t[:], in_=simP[:])
                nc.sync.dma_start(out=dbg["d_sim"][:], in_=t[:])

            # ---- 13+14. softmax (deferred normalization: the head-expand of
            # the unnormalized exp runs on PE while sum/reciprocal run on DVE)
            ex = pool.tile([8, 288], BF16, name="ex")
            nc.scalar.activation(out=ex[:], in_=simP[:], func=AF.Exp)
            smt = pool.tile([8, 36], F32, name="smt")
            nc.vector.reduce_sum(out=smt[:], in_=view3(ex[:], [[1, 36], [36, 8]]),
                                 axis=mybir.AxisListType.X)
            rct = pool.tile([8, 36], BF16, name="rct")
            with nc.allow_low_precision(reason="bf16 softmax norm is well "
                                        "within the 2e-2 tolerance"):
                nc.vector.reciprocal(out=rct[:], in_=smt[:])
            if debug:
                exn = pool.tile([8, 288], BF16, name="exn")
                rca = rct[:]
                nc.vector.tensor_tensor(
                    out=view3(exn[:], [[1, 36], [36, 8]]),
                    in0=view3(ex[:], [[1, 36], [36, 8]]),
                    in1=AP(rca.tensor, rca.offset, [rca.ap[0], [1, 36], [0, 8]]),
                    op=OP.mult)
                nc.sync.dma_start(out=dbg["d_at"][:], in_=exn[:])

            avT = []
            aeP, reP, avu = [], [], []
            for fc in range(2):
                ae = psum.tile([128, 288], F32, space="PSUM", tag="psA",
                               bufs=3, name=f"aeP{fc}")
                nc.tensor.matmul(out=ae[:], lhsT=wfs(f"e{fc}", rows=8,
                                                     width=128),
                                 rhs=ex[:], start=True, stop=True)
                aeP.append(ae)
            for fc in range(2):
                re = psum.tile([128, 288], F32, space="PSUM", tag="psA",
                               bufs=3, name=f"reP{fc}")
                nc.tensor.matmul(out=re[:, :36], lhsT=wfs(f"e{fc}", rows=8,
                                                          width=128),
                                 rhs=rct[:], start=True, stop=True)
                reP.append(re)
            for fc in range(2):
                pr = pool.tile([128, 288], BF16, name=f"pr{fc}")
                nc.vector.tensor_tensor(out=pr[:], in0=vS[fc][:],
                                        in1=aeP[fc][:], op=OP.mult)
                au = pool.tile([128, 36], BF16, name=f"avu{fc}")
                with nc.allow_low_precision(reason="bf16 attn output is well "
                                            "within the 2e-2 tolerance"):
                    nc.vector.reduce_sum(out=au[:],
                                         in_=view3(pr[:], [[1, 36], [36, 8]]),
                                         axis=mybir.AxisListType.X)
                av = pool.tile([128, 36], BF16, name=f"avT{fc}")
                nc.vector.tensor_tensor(out=av[:], in0=au[:],
                                        in1=reP[fc][:, :36], op=OP.mult)
                avT.append(av)
            if debug:
                nc.sync.dma_start(out=dbg["d_av0"][:], in_=avT[0][:])

            # ---- 15. out = attn_out @ W_out + b_out + identity
            oP = psum.tile([128, 288], F32, space="PSUM", tag="psA", bufs=3,
                           name="oP")
            oT = pool.tile([128, 72], F32, name="oT")
            oda = out[:]
            for mc in range(2):
                for kc in range(2):
                    nc.tensor.matmul(
                        out=oP[:, mc * 36:(mc + 1) * 36],
                        lhsT=wfs("wout", off=(kc * 2 + mc) * 128, width=128),
                        rhs=avT[kc][:], start=(kc == 0), stop=(kc == 1))
                nc.scalar.activation(out=oT[:, mc * 36:(mc + 1) * 36],
                                     in_=oP[:, mc * 36:(mc + 1) * 36],
                                     func=AF.Identity,
                                     bias=wgs("bout", off=mc, width=1))
                nc.vector.tensor_tensor(out=oT[:, mc * 36:(mc + 1) * 36],
                                        in0=oT[:, mc * 36:(mc + 1) * 36],
                                        in1=xbs("deT", off=mc * 36, width=36),
                                        op=OP.add)
                ota = oT[:, mc * 36:(mc + 1) * 36]
                nc.sync.dma_start(
                    out=AP(oda.tensor, oda.offset + mc * 128 * 36,
                           [[36, 128], [1, 36]]),
                    in_=AP(ota.tensor, ota.offset, [[72, 128], [1, 36]]))

    return nc


# ------------------------------------------------------------------- driver

def make_in_maps(dec_embed, bev_feat, query_scale, ref_points, weights):
    wf = pack_wf(weights)
    wg = pack_wg(weights)
    bevs = []
    for b in range(B):
        hwc = bev_feat[b].transpose(1, 2, 0).reshape(H * W, 256)
        bev_hwc = np.zeros((H * W, 512), np.float32)
        bev_hwc[:, 0:256] = hwc
        bev_hwc[:(H - 1) * W, 256:512] = hwc[W:]
        bevs.append(np.ascontiguousarray(bev_hwc.astype(NPBF)))
    in_maps = []
    for c in range(8):
        b, kh = c // 2, c % 2
        in_maps.append({
            "bev": bevs[b], "wf": wf, "wg": wg,
            "xbl": pack_xb(dec_embed, query_scale, ref_points, b, 3 * kh),
            "xh": pack_xh(dec_embed, b, 3 * kh),
        })
    return in_maps


def assemble_output(results):
    out = np.zeros((K, B, T, DIM), np.float32)
    for c in range(8):
        b, kh = c // 2, c % 2
        oc = results[c]["out"]                     # (256, 36)
        out[3 * kh:3 * kh + 3, b] = oc.T.reshape(3, T, DIM)
    return out


_WNAMES = ["W_con_q", "b_con_q", "W_con_k", "W_v", "Wq1", "bq1", "Wq2", "bq2",
           "Wk1", "bk1", "Wk2", "bk2", "Wo1", "bo1", "Wo2", "bo2",
           "W_out", "b_out"]


def kernel(**inputs):
    from concourse.bass_utils import run_bass_kernel_spmd
    dec_embed = np.asarray(inputs["dec_embed"], np.float32)
    bev_feat = np.asarray(inputs["bev_feat"], np.float32)
    query_scale = np.asarray(inputs["query_scale"], np.float32)
    ref_points = np.asarray(inputs["ref_points"], np.float32)
    weights = {n: np.asarray(inputs[n], np.float32) for n in _WNAMES}

    nc = build_nc(sim_mode=False, debug=False)
    split_multiwaits(nc)
    in_maps = make_in_maps(dec_embed, bev_feat, query_scale, ref_points, weights)
    res = run_bass_kernel_spmd(nc, in_maps, list(range(8)))
    return assemble_output(res.results)


# revision 12
# speedup vs baseline: 1.6152x; 1.0237x over previous
"""BEV deformable cross-attention kernel for 8 Trainium2 NeuronCores.

Strategy (per core): data-parallel over (B x K-half): core c handles batch
b = c//2 and modes k in {3*(c%2) .. +3}, i.e. 36 queries, 288 sample points.

Key algebraic move: grid_sample(conv1x1(bev)) == conv1x1(grid_sample(bev)),
so instead of materializing the two full (256,200,200) conv maps we gather
only the 4 bilinear corners of the 288 sample points from a host-transposed
HWC copy of bev_feat (channels contiguous per pixel -> 2KB indirect reads),
interpolate in 256-d, then apply the 1x1 convs to 288 vectors.

v2 perf notes vs the first working version:
- all fat matmuls run in bf16 (fp32 matmuls lower to 2 half-rate HW passes);
  only the sine-phase matmuls and the geometry path stay fp32.
- weights/inputs land via parallel DMA queues (gpsimd + sync) so the first
  matmul no longer waits on unrelated blobs.
- gelu is computed through the Silu table (gelu(x) ~ x*sigmoid(1.702x),
  exact for the tiny pre-activations here) so {silu,tanh,sin} share one
  activation table and only one mid-kernel table switch (exp) remains.
- the bilinear combine is 4 fused scalar_tensor_tensor ops per chunk on the
  Pool engine; softmax normalizes before head-expansion (no re-expand mm).
"""
import numpy as np
import ml_dtypes

import concourse.bass as bass
import concourse.mybir as mybir
import concourse.tile as tile_mod
from concourse.bass import AP, IndirectOffsetOnAxis

F32 = mybir.dt.float32
BF16 = mybir.dt.bfloat16
I32 = mybir.dt.int32
AF = mybir.ActivationFunctionType
OP = mybir.AluOpType
NPBF = ml_dtypes.bfloat16

# problem constants (hardcoded per contract)
K, B, T, DIM = 6, 4, 12, 256
H, W = 200, 200
HALF = 256
G = 8                      # offset groups == sample points per query
NQ = 3 * T                 # queries per core = 36
NPT = NQ * G               # points per core = 288
OFFSET_SCALE = 4.0
PIX_SCALE = float(W / 102.4)          # 1.953125
PIX_BIAS = float(W / 2.0 - 0.5)       # 99.5
SCALE = 64 ** -0.5                    # 0.125
TWO_PI = float(2 * np.pi)
RC = float(3 * 2 ** 22)               # 1.5*2^23 rint magic constant
SILU_A = 1.702                        # gelu(x) ~ silu(1.702 x)/1.702
CHUNKS = [(0, 128), (128, 128), (256, 32)]   # point chunks (start, size)

# ---------------------------------------------------------------- blob layout


class Alloc:
    def __init__(self):
        self.pos = 0
        self.slices = {}

    def add(self, name, width):
        self.slices[name] = (self.pos, width)
        self.pos += width

    def __getitem__(self, name):
        return self.slices[name]


# bf16 matmul-weight blob; split points F0/F1/F2 are separate DMAs so the
# critical-path prefix (wconq) lands first.
WF_ITEMS = [("wconq", 512), ("bdh", 512), ("wo2t", 2), ("wo2b", 2),  # F0
            ("wq1", 512), ("wq2", 512),                                # F1
            ("s0", 8), ("s1", 8), ("e0", 128), ("e1", 128), ("identB", 128),
            ("wk1", 512), ("wk2", 512), ("wcat", 1024), ("wout", 512)]  # F2
F0_END = 1028
F1_END = 1028 + 1024 + 16 + 256 + 128              # 2452

# fp32 misc blob: biases (as (128,2) column pairs), geometry consts,
# sine-phase weights.
WG_ITEMS = [("bconq", 2), ("bo1s", 1), ("bo2", 1), ("sc4pm", 2),
            ("fq2", 128), ("fk5x", 128), ("fk5y", 128), ("id2", 2),
            ("bq1", 2), ("bq2", 2), ("bk1", 2), ("bk2", 2), ("bout", 2)]

# fp32 per-core input blob. rpo holds [tanh_x; tanh_y; rpx; rpy; ones] rows:
# partitions 0:2 are blank (filled by the on-device tanh), 2:5 host data.
XB_ITEMS = [("deT", 72), ("qsT", 72), ("rpyx1", 72), ("rpo", 288), ("bpm", 6)]


def _layout(items):
    a = Alloc()
    for nm, wd in items:
        a.add(nm, wd)
    return a


WF_LAY = _layout(WF_ITEMS)
WG_LAY = _layout(WG_ITEMS)
XB_LAY = _layout(XB_ITEMS)


def _put_mm(dst, lay, name, w256):
    """(256, Mout) -> (kc, mc) blocks of (128, 128) at s + (kc*mcs+mc)*128."""
    s, _ = lay[name]
    mcs = w256.shape[1] // 128
    for kc in range(2):
        for mc in range(mcs):
            blk = w256[kc * 128:(kc + 1) * 128, mc * 128:(mc + 1) * 128]
            off = (kc * mcs + mc) * 128
            dst[:, s + off: s + off + 128] = blk


def pack_wf(weights):
    wf = np.zeros((128, WF_LAY.pos), np.float32)
    lay = WF_LAY

    def put(name, arr, rows=128):
        s, _ = lay[name]
        wf[:rows, s: s + arr.shape[1]] = arr

    _put_mm(wf, lay, "wconq", weights["W_con_q"])
    # block-diag Wo1: block j covers groups (2j, 2j+1); even j from feature
    # chunk 0 rows, odd j from chunk 1 rows.
    s, _ = lay["bdh"]
    wo1 = weights["Wo1"]  # (32, 64)
    for j in range(4):
        blk = np.zeros((128, 128), np.float32)
        if j % 2 == 0:
            blk[0:32, 0:64] = wo1
            blk[32:64, 64:128] = wo1
        else:
            blk[64:96, 0:64] = wo1
            blk[96:128, 64:128] = wo1
        wf[:, s + j * 128: s + (j + 1) * 128] = blk
    wo2 = weights["Wo2"] / SILU_A          # undo the silu input scale
    top = np.zeros((128, 2), np.float32); top[0:64] = wo2
    bot = np.zeros((128, 2), np.float32); bot[64:128] = wo2
    put("wo2t", top); put("wo2b", bot)
    _put_mm(wf, lay, "wq1", weights["Wq1"])
    _put_mm(wf, lay, "wq2", weights["Wq2"])
    _put_mm(wf, lay, "wk1", weights["Wk1"])
    _put_mm(wf, lay, "wk2", weights["Wk2"])
    wcat = np.concatenate([weights["W_con_k"], weights["W_v"]], axis=1)
    _put_mm(wf, lay, "wcat", wcat)
    _put_mm(wf, lay, "wout", weights["W_out"])
    d = np.arange(128)
    s0 = np.zeros((128, 8), np.float32); s0[d, d // 32] = SCALE
    s1 = np.zeros((128, 8), np.float32); s1[d, 4 + d // 32] = SCALE
    put("s0", s0); put("s1", s1)
    e0 = np.zeros((8, 128), np.float32); e0[d // 32, d] = 1.0
    e1 = np.zeros((8, 128), np.float32); e1[4 + d // 32, d] = 1.0
    put("e0", e0, rows=8); put("e1", e1, rows=8)
    put("identB", np.eye(128, dtype=np.float32))
    return wf.astype(NPBF)


def _freq_shift():
    i64 = np.arange(128) // 2
    freq = (TWO_PI / (10000.0 ** (i64 / 64.0))).astype(np.float32)
    shift = np.where(np.arange(128) % 2 == 1, np.pi / 2, 0.0).astype(np.float32)
    return freq, shift


def pack_wg(weights):
    wg = np.zeros((128, WG_LAY.pos), np.float32)
    lay = WG_LAY

    def put(name, arr, rows=128):
        s, _ = lay[name]
        wg[:rows, s: s + arr.shape[1]] = arr

    put("bconq", weights["b_con_q"].reshape(2, 128).T)
    put("bo1s", SILU_A * np.tile(weights["bo1"], 2)[:, None])
    put("bo2", weights["bo2"][:, None], rows=2)
    put("sc4pm", np.tile(np.array([[4 * PIX_SCALE, -4 * PIX_SCALE]],
                                  np.float32), (128, 1)))
    freq, shift = _freq_shift()
    put("fq2", np.stack([freq, shift]), rows=2)
    fk5x = np.zeros((5, 128), np.float32)
    fk5x[0] = 4 * freq; fk5x[2] = freq; fk5x[4] = shift
    fk5y = np.zeros((5, 128), np.float32)
    fk5y[1] = 4 * freq; fk5y[3] = freq; fk5y[4] = shift
    put("fk5x", fk5x, rows=5)
    put("fk5y", fk5y, rows=5)
    put("id2", np.eye(2, dtype=np.float32), rows=2)
    put("bq1", weights["bq1"].reshape(2, 128).T)
    put("bq2", weights["bq2"].reshape(2, 128).T)
    put("bk1", weights["bk1"].reshape(2, 128).T)
    put("bk2", weights["bk2"].reshape(2, 128).T)
    put("bout", weights["b_out"].reshape(2, 128).T)
    return wg


def pack_xb(dec_embed, query_scale, ref_points, b, k0):
    lay = XB_LAY
    xb = np.zeros((128, lay.pos), np.float32)
    de = dec_embed[k0:k0 + 3, b].reshape(NQ, DIM)       # (36, 256)
    qs = query_scale[k0:k0 + 3, b].reshape(NQ, DIM)
    rp = ref_points[k0:k0 + 3, b].reshape(NQ, 2)

    s, _ = lay["deT"]
    xb[:, s: s + 36] = de.T[:128]
    xb[:, s + 36: s + 72] = de.T[128:]
    s, _ = lay["qsT"]
    xb[:, s: s + 36] = qs.T[:128]
    xb[:, s + 36: s + 72] = qs.T[128:]
    s, _ = lay["rpyx1"]
    xb[0, s: s + 36] = rp[:, 1]                         # y first (DAB order)
    xb[0, s + 36: s + 72] = rp[:, 0]
    xb[1, s: s + 72] = 1.0
    s, _ = lay["rpo"]
    rpe = np.tile(rp.T, (1, 8))                         # g-major: col = g*36+q
    xb[2, s: s + 288] = rpe[0]
    xb[3, s: s + 288] = rpe[1]
    xb[4, s: s + 288] = 1.0
    s, _ = lay["bpm"]
    bx = PIX_SCALE * rpe[0] + PIX_BIAS
    by = -PIX_SCALE * rpe[1] + PIX_BIAS
    for c, (c0, cn) in enumerate(CHUNKS):
        xb[:cn, s + 2 * c] = bx[c0:c0 + cn]
        xb[:cn, s + 2 * c + 1] = by[c0:c0 + cn]
    return xb


def pack_xh(dec_embed, b, k0):
    de = dec_embed[k0:k0 + 3, b].reshape(NQ, DIM)
    xh = np.zeros((128, 72), np.float32)
    xh[:, 0:36] = de.T[:128]
    xh[:, 36:72] = de.T[128:]
    return xh.astype(NPBF)


# --------------------------------------------------------------- tile patches

def _split_drain_and_barrier(self, tick_clock, wait_clock):
    nc = self.nc
    drain_inst = nc.sync.drain()
    wait_clock.add_sem_waits(
        drain_inst.ins, tile_mod.ScopedClock({None: tick_clock.global_clock})
    )
    si = drain_inst.ins.sync_info
    waits = list(si.on_wait)
    if len(waits) > 1:
        si.on_wait = waits[:1]
        for i in range(1, len(waits)):
            extra = nc.sync.drain()
            extra.ins.sync_info = type(si)(on_wait=waits[i: i + 1], on_update=[])
    nc.all_engine_barrier()
    assert self.sems is not None
    popped = nc._tile_sem_poison_stack.pop()
    assert popped is self._sem_poison
    nc.clear_and_free_semaphores(list(self.sems.allocated().values()))


def split_multiwaits(nc):
    """walrus codegen supports a single sync-wait per instruction; split."""
    f = nc.m.functions[0]
    for blk in f.blocks:
        todo = [i for i in blk.instructions
                if i.sync_info is not None and len(i.sync_info.on_wait) > 1]
        for inst in todo:
            si = inst.sync_info
            waits = list(si.on_wait)
            nops = []
            for w in waits[:-1]:
                bi = nc.engines[inst.engine].nop(nofuse=True)
                ni = bi.ins
                for b2 in f.blocks:
                    if b2.instructions and b2.instructions[-1] is ni:
                        b2.instructions.pop()
                        break
                ni.sync_info = type(si)(on_wait=[w], on_update=[])
                nops.append(ni)
            si.on_wait = [waits[-1]]
            pos = blk.instructions.index(inst)
            blk.instructions[pos:pos] = nops


_PATCHED = False


def patch_tile():
    global _PATCHED
    if not _PATCHED:
        tile_mod.TileContext._drain_and_barrier = _split_drain_and_barrier
        _PATCHED = True


# ---------------------------------------------------------------- the kernel

def view3(ap, dims):
    """3D AP view over a 2D tile AP: dims = [[step,count],...] after ap[0]."""
    return AP(ap.tensor, ap.offset, [ap.ap[0]] + dims)


def build_nc(sim_mode=False, debug=False):
    patch_tile()
    nc = bass.Bass("TRN2")

    # row-pair interleaved bf16: bev[y*W+x] = [feat(y,x) | feat(y+1,x)]
    bev = nc.dram_tensor("bev", [H * W, 512], BF16, kind="ExternalInput")
    wfD = nc.dram_tensor("wf", [128, WF_LAY.pos], BF16, kind="ExternalInput")
    wgD = nc.dram_tensor("wg", [128, WG_LAY.pos], F32, kind="ExternalInput")
    xbD = nc.dram_tensor("xbl", [128, XB_LAY.pos], F32, kind="ExternalInput")
    xhD = nc.dram_tensor("xh", [128, 72], BF16, kind="ExternalInput")
    out = nc.dram_tensor("out", [256, NQ], F32, kind="ExternalOutput")

    dbg = {}
    if debug:
        for nm, shp, dt in [
            ("d_pix", [128, 2], F32), ("d_idx", [128, 1], I32),
            ("d_w40", [128, 4], F32), ("d_sam0", [128, 256], BF16),
            ("d_cq0", [128, 36], BF16), ("d_h", [128, 144], BF16),
            ("d_qse0", [128, 36], BF16), ("d_kse0", [128, 288], BF16),
            ("d_posk0", [128, 288], BF16), ("d_conv0", [128, 288], F32),
            ("d_sim", [8, 288], F32), ("d_at", [8, 288], BF16),
            ("d_av0", [128, 36], BF16),
        ]:
            dbg[nm] = nc.dram_tensor(nm, shp, dt, kind="ExternalOutput")

    with tile_mod.TileContext(nc) as tc:
        with (
            tc.tile_pool(name="sbuf", bufs=1) as pool,
            tc.tile_pool(name="psum", bufs=1, space="PSUM") as psum,
        ):
            # warm the {silu,tanh,sin} table during the input DMAs
            wt = pool.tile([1, 1], F32)
            nc.vector.memset(wt[:], 0.0)
            warm = pool.tile([1, 1], F32)
            nc.scalar.activation(out=warm[:], in_=wt[:],
                                 func=AF.Sigmoid if sim_mode else AF.Silu,
                                 bias=0.0)

            # ---- input DMAs.  wf is three separate tiles so the con_q
            # matmul only waits on the wconq prefix, not the whole blob.
            xh = pool.tile([128, 72], BF16)
            nc.sync.dma_start(out=xh[:], in_=xhD[:])
            wf0 = pool.tile([128, F0_END], BF16)
            nc.sync.dma_start(out=wf0[:], in_=wfD[:, 0:F0_END])
            wg = pool.tile([128, WG_LAY.pos], F32)
            nc.sync.dma_start(out=wg[:], in_=wgD[:])
            xb = pool.tile([128, XB_LAY.pos], F32)
            nc.gpsimd.dma_start(out=xb[:], in_=xbD[:])
            wf1 = pool.tile([128, F1_END - F0_END], BF16)
            nc.gpsimd.dma_start(out=wf1[:], in_=wfD[:, F0_END:F1_END])
            wf2 = pool.tile([128, WF_LAY.pos - F1_END], BF16)
            nc.gpsimd.dma_start(out=wf2[:], in_=wfD[:, F1_END:WF_LAY.pos])

            def wfs(name, rows=128, off=0, width=None):
                s, wd = WF_LAY[name]
                if width is None:
                    width = wd - off
                if s < F0_END:
                    t, base = wf0, 0
                elif s < F1_END:
                    t, base = wf1, F0_END
                else:
                    t, base = wf2, F1_END
                return t[0:rows, s - base + off: s - base + off + width]

            def wgs(name, rows=128, off=0, width=None):
                s, wd = WG_LAY[name]
                if width is None:
                    width = wd - off
                return wg[0:rows, s + off: s + off + width]

            def xbs(name, rows=128, off=0, width=None):
                s, wd = XB_LAY[name]
                if width is None:
                    width = wd - off
                return xb[0:rows, s + off: s + off + width]

            # ---- 1. con_q = de @ W_con_q + b   (bf16)
            cqP = psum.tile([128, 288], F32, space="PSUM", tag="psA", bufs=3,
                            name="cqP")
            for mc in range(2):
                for kc in range(2):
                    nc.tensor.matmul(
                        out=cqP[:, mc * 36:(mc + 1) * 36],
                        lhsT=wfs("wconq", off=(kc * 2 + mc) * 128, width=128),
                        rhs=xh[:, kc * 36:(kc + 1) * 36],
                        start=(kc == 0), stop=(kc == 1))
            cqS = pool.tile([128, 72], BF16, name="cqS")
            for mc in range(2):
                nc.scalar.activation(out=cqS[:, mc * 36:(mc + 1) * 36],
                                     in_=cqP[:, mc * 36:(mc + 1) * 36],
                                     func=AF.Identity,
                                     bias=wgs("bconq", off=mc, width=1))
            if debug:
                nc.sync.dma_start(out=dbg["d_cq0"][:], in_=cqS[:, 0:36])

            # ---- 2. h = gelu(grouped con_q @ Wo1 + bo1) via silu table
            hP = psum.tile([128, 288], F32, space="PSUM", tag="psA", bufs=3,
                           name="hP")
            for j in range(4):
                cc = j // 2
                nc.tensor.matmul(
                    out=hP[:, j * 36:(j + 1) * 36],
                    lhsT=wfs("bdh", off=j * 128, width=128),
                    rhs=cqS[:, cc * 36:(cc + 1) * 36], start=True, stop=True)
            hS = pool.tile([128, 144], BF16, name="hS")
            if sim_mode:
                hx = pool.tile([128, 144], F32)
                nc.scalar.activation(out=hx[:], in_=hP[:, :144],
                                     func=AF.Identity, scale=SILU_A,
                                     bias=wgs("bo1s"))
                he = pool.tile([128, 144], F32)
                nc.scalar.activation(out=he[:], in_=hx[:], func=AF.Sigmoid,
                                     bias=0.0)
                nc.vector.tensor_tensor(out=hS[:], in0=hx[:], in1=he[:],
                                        op=OP.mult)
            else:
                nc.scalar.activation(out=hS[:], in_=hP[:, :144], func=AF.Silu,
                                     scale=SILU_A, bias=wgs("bo1s"))
            if debug:
                nc.sync.dma_start(out=dbg["d_h"][:], in_=hS[:])

            # ---- 3. offsets -> tanh into xb rows 0:2 of the rpo region
            offP = psum.tile([2, 288], F32, space="PSUM", tag="psA", bufs=3,
                             name="offP")
            for m, wn in [(0, "wo2t"), (1, "wo2b")]:
                nc.tensor.matmul(
                    out=offP[:, m * 144:(m + 1) * 144],
                    lhsT=wfs(wn, width=2), rhs=hS[:], start=True, stop=True)
            # qse phase matmul slotted here: fills the PE bubble while the
            # tanh runs on ACT.
            phQ = psum.tile([128, 288], F32, space="PSUM", tag="psA", bufs=3,
                            name="phQ")
            nc.tensor.matmul(out=phQ[:, :72], lhsT=wgs("fq2", rows=2),
                             rhs=xbs("rpyx1", rows=2), start=True, stop=True)
            s_rpo, _ = XB_LAY["rpo"]
            kra = xb[0:2, s_rpo:s_rpo + 288]
            opa = offP[:]
            nc.scalar.activation(
                out=AP(kra.tensor, kra.offset,
                       [kra.ap[0], [72, 4], [36, 2], [1, 36]]),
                in_=AP(opa.tensor, opa.offset,
                       [opa.ap[0], [36, 4], [144, 2], [1, 36]]),
                func=AF.Tanh, bias=wgs("bo2", rows=2, width=1))
            kseRhs = xb[0:5, s_rpo:s_rpo + 288]

            # ---- 4. per-chunk geometry -> indices -> gathers (bf16 rows)
            s_bpm, _ = XB_LAY["bpm"]
            frs, idxI, gA, w4 = [], [], [], []
            pix0 = None
            for c, (c0, cn) in enumerate(CHUNKS):
                tp = psum.tile([128, 2], F32, space="PSUM", tag="psA", bufs=3,
                               name=f"tpP{c}")
                nc.tensor.transpose(out=tp[:cn, :], in_=kseRhs[0:2, c0:c0 + cn],
                                    identity=wgs("id2", rows=2, width=2))
                pix = pool.tile([128, 2], F32, name=f"pix{c}")
                if c == 0:
                    pix0 = pix
                nc.vector.tensor_tensor(out=pix[:cn, :], in0=tp[:cn, :],
                                        in1=wgs("sc4pm", rows=cn, width=2),
                                        op=OP.mult)
                nc.vector.tensor_tensor(
                    out=pix[:cn, :], in0=pix[:cn, :],
                    in1=xb[0:cn, s_bpm + 2 * c: s_bpm + 2 * c + 2], op=OP.add)
                f0 = pool.tile([128, 2], F32, name=f"f0{c}")
                nc.vector.tensor_scalar(out=f0[:cn, :], in0=pix[:cn, :],
                                        scalar1=-0.5, scalar2=float(RC),
                                        op0=OP.add, op1=OP.add)
                nc.vector.tensor_scalar(out=f0[:cn, :], in0=f0[:cn, :],
                                        scalar1=float(-RC), scalar2=None,
                                        op0=OP.add)
                fr = pool.tile([128, 2], F32, name=f"fr{c}")
                nc.vector.tensor_tensor(out=fr[:cn, :], in0=pix[:cn, :],
                                        in1=f0[:cn, :], op=OP.subtract)
                frs.append(fr)
                idf = pool.tile([128, 1], F32, name=f"idf{c}")
                nc.vector.scalar_tensor_tensor(
                    out=idf[:cn, :], in0=f0[:cn, 1:2], scalar=float(W),
                    in1=f0[:cn, 0:1], op0=OP.mult, op1=OP.add)
                ii = pool.tile([128, 1], I32, name=f"idxI{c}")
                nc.vector.tensor_copy(out=ii[:cn, :], in_=idf[:cn, :])
                idxI.append(ii)
                ga = pool.tile([128, 1024], BF16, name=f"gA{c}")
                nc.gpsimd.indirect_dma_start(
                    out=ga[:cn, :], out_offset=None, in_=bev[:],
                    in_offset=IndirectOffsetOnAxis(ap=ii[:cn, :], axis=0))
                gA.append(ga)
            # bilinear weights (Pc, 4) = [w00, w10, w01, w11]
            for c, (c0, cn) in enumerate(CHUNKS):
                fr = frs[c]
                wxp = pool.tile([128, 2], F32, name=f"wxp{c}")
                nc.vector.tensor_scalar(out=wxp[:cn, 0:1], in0=fr[:cn, 0:1],
                                        scalar1=-1.0, scalar2=1.0,
                                        op0=OP.mult, op1=OP.add)
                nc.vector.tensor_copy(out=wxp[:cn, 1:2], in_=fr[:cn, 0:1])
                wyp = pool.tile([128, 2], F32, name=f"wyp{c}")
                nc.vector.tensor_scalar(out=wyp[:cn, 0:1], in0=fr[:cn, 1:2],
                                        scalar1=-1.0, scalar2=1.0,
                                        op0=OP.mult, op1=OP.add)
                nc.vector.tensor_copy(out=wyp[:cn, 1:2], in_=fr[:cn, 1:2])
                w4c = pool.tile([128, 4], F32, name=f"w4{c}")
                wxa = wxp[:cn, :]
                wya = wyp[:cn, :]
                nc.vector.tensor_tensor(
                    out=view3(w4c[:cn, :], [[2, 2], [1, 2]]),
                    in0=AP(wxa.tensor, wxa.offset, [wxa.ap[0], [0, 2], [1, 2]]),
                    in1=AP(wya.tensor, wya.offset, [wya.ap[0], [1, 2], [0, 2]]),
                    op=OP.mult)
                w4.append(w4c)
            if debug:
                nc.sync.dma_start(out=dbg["d_pix"][:], in_=pix0[:])
                nc.sync.dma_start(out=dbg["d_idx"][:], in_=idxI[0][:])
                nc.sync.dma_start(out=dbg["d_w40"][:], in_=w4[0][:])

            # ---- 5. kse phase matmuls (fp32) right after the tp transposes:
            # they only need the tanh, and fill the PE gather window.
            phK = []
            for ax, wn in [(0, "fk5y"), (1, "fk5x")]:
                p = psum.tile([128, 288], F32, space="PSUM", tag="psA",
                              bufs=3, name=f"phK{ax}")
                nc.tensor.matmul(out=p[:], lhsT=wgs(wn, rows=5),
                                 rhs=kseRhs, start=True, stop=True)
                phK.append(p)

            # ---- 6. qse sin (range reduce on DVE)
            qse = pool.tile([128, 72], BF16, name="qse")
            m1q = pool.tile([128, 72], F32, name="m1q")
            nc.vector.tensor_scalar(out=m1q[:], in0=phQ[:, :72],
                                    scalar1=float(1.0 / TWO_PI),
                                    scalar2=RC, op0=OP.mult, op1=OP.add)
            nc.vector.tensor_scalar(out=m1q[:], in0=m1q[:], scalar1=-RC,
                                    scalar2=-TWO_PI, op0=OP.add, op1=OP.mult)
            ytq = pool.tile([128, 72], F32, name="ytq")
            nc.vector.tensor_tensor(out=ytq[:], in0=phQ[:, :72], in1=m1q[:],
                                    op=OP.add)
            nc.vector.tensor_scalar(out=ytq[:], in0=ytq[:],
                                    scalar1=float(np.pi),
                                    scalar2=float(-np.pi),
                                    op0=OP.min, op1=OP.max)
            nc.scalar.activation(out=qse[:], in_=ytq[:], func=AF.Sin)
            if debug:
                nc.sync.dma_start(out=dbg["d_qse0"][:], in_=qse[:, 0:36])

            # ---- 7a. pos_q MLP layer 1 matmuls (bf16, in the gather window)
            mqP = psum.tile([128, 288], F32, space="PSUM", tag="psA", bufs=3,
                            name="mqP")
            for mc in range(2):
                for kc in range(2):
                    nc.tensor.matmul(
                        out=mqP[:, mc * 36:(mc + 1) * 36],
                        lhsT=wfs("wq1", off=(kc * 2 + mc) * 128, width=128),
                        rhs=qse[:, kc * 36:(kc + 1) * 36],
                        start=(kc == 0), stop=(kc == 1))

            # ---- 8. kse sins (m1 on ACT, k2/y/clip on DVE), each axis
            # followed by a bilinear-combine chunk on DVE (readiness order).
            def kse_axis(ax):
                m1 = pool.tile([128, 288], F32, name=f"m1k{ax}")
                nc.scalar.activation(out=m1[:], in_=phK[ax][:], func=AF.Copy,
                                     scale=float(1.0 / TWO_PI), bias=float(RC))
                nc.vector.tensor_scalar(out=m1[:], in0=m1[:], scalar1=-RC,
                                        scalar2=-TWO_PI, op0=OP.add,
                                        op1=OP.mult)
                yt = pool.tile([128, 288], F32, name=f"ytk{ax}")
                nc.vector.tensor_tensor(out=yt[:], in0=phK[ax][:], in1=m1[:],
                                        op=OP.add)
                nc.vector.tensor_scalar(out=yt[:], in0=yt[:],
                                        scalar1=float(np.pi),
                                        scalar2=float(-np.pi),
                                        op0=OP.min, op1=OP.max)
                st = pool.tile([128, 288], BF16, name=f"kse{ax}")
                nc.scalar.activation(out=st[:], in_=yt[:], func=AF.Sin)
                return st

            # combine: gather quarters [c00|c01|c10|c11]; quarter j uses w4
            # col [0, 2, 1, 3][j].
            sam = [None, None, None]

            def combine_chunk(c):
                c0, cn = CHUNKS[c]
                g = gA[c]
                t1 = pool.tile([128, 256], BF16, name=f"bt{c}")
                sm = pool.tile([128, 256], BF16, name=f"sam{c}")
                nc.vector.tensor_scalar(out=t1[:cn, :], in0=g[:cn, 0:256],
                                        scalar1=w4[c][:cn, 0:1], scalar2=None,
                                        op0=OP.mult)
                nc.vector.scalar_tensor_tensor(
                    out=t1[:cn, :], in0=g[:cn, 256:512],
                    scalar=w4[c][:cn, 2:3], in1=t1[:cn, :],
                    op0=OP.mult, op1=OP.add)
                nc.vector.scalar_tensor_tensor(
                    out=t1[:cn, :], in0=g[:cn, 512:768],
                    scalar=w4[c][:cn, 1:2], in1=t1[:cn, :],
                    op0=OP.mult, op1=OP.add)
                nc.vector.scalar_tensor_tensor(
                    out=sm[:cn, :], in0=g[:cn, 768:1024],
                    scalar=w4[c][:cn, 3:4], in1=t1[:cn, :],
                    op0=OP.mult, op1=OP.add)
                sam[c] = sm

            kse = [None, None]
            kse[0] = kse_axis(0)
            combine_chunk(0)
            kse[1] = kse_axis(1)
            combine_chunk(1)
            if debug:
                nc.sync.dma_start(out=dbg["d_kse0"][:], in_=kse[0][:])
                nc.sync.dma_start(out=dbg["d_sam0"][:], in_=sam[0][:])

            # ---- 10+11. PE: transposes interleaved with the pos_k MLP and
            # pos_q layer 2, ordered by operand readiness.
            samTP = []
            for fc in range(2):
                samTP.append(psum.tile([128, 288], BF16, space="PSUM",
                                       tag="psA", bufs=3, name=f"samTP{fc}"))

            def transpose_chunk(c):
                c0, cn = CHUNKS[c]
                for fc in range(2):
                    nc.tensor.transpose(
                        out=samTP[fc][:, c0:c0 + cn],
                        in_=sam[c][:cn, fc * 128:(fc + 1) * 128],
                        identity=wfs("identB", rows=cn, width=cn))

            transpose_chunk(0)
            midK = []
            for mc in range(2):
                p = psum.tile([128, 288], F32, space="PSUM", tag="psA", bufs=3,
                              name=f"mkP{mc}")
                for kc in range(2):
                    nc.tensor.matmul(
                        out=p[:], lhsT=wfs("wk1", off=(kc * 2 + mc) * 128,
                                           width=128),
                        rhs=kse[kc][:], start=(kc == 0), stop=(kc == 1))
                t = pool.tile([128, 288], BF16, name=f"midK{mc}")
                nc.scalar.activation(out=t[:], in_=p[:], func=AF.Relu,
                                     bias=wgs("bk1", off=mc, width=1))
                midK.append(t)

            # pos_q tail: midQ relu + layer2 + bias*scale (DVE after combine)
            midQ = pool.tile([128, 72], BF16, name="midQ")
            for mc in range(2):
                nc.vector.tensor_scalar(
                    out=midQ[:, mc * 36:(mc + 1) * 36],
                    in0=mqP[:, mc * 36:(mc + 1) * 36],
                    scalar1=wgs("bq1", off=mc, width=1), scalar2=0.0,
                    op0=OP.add, op1=OP.max)
            pqP = psum.tile([128, 288], F32, space="PSUM", tag="psA", bufs=3,
                            name="pqP")
            for mc in range(2):
                for kc in range(2):
                    nc.tensor.matmul(
                        out=pqP[:, mc * 36:(mc + 1) * 36],
                        lhsT=wfs("wq2", off=(kc * 2 + mc) * 128, width=128),
                        rhs=midQ[:, kc * 36:(kc + 1) * 36],
                        start=(kc == 0), stop=(kc == 1))
            pqS = pool.tile([128, 72], BF16, name="pqS")
            for mc in range(2):
                nc.vector.scalar_tensor_tensor(
                    out=pqS[:, mc * 36:(mc + 1) * 36],
                    in0=pqP[:, mc * 36:(mc + 1) * 36],
                    scalar=wgs("bq2", off=mc, width=1),
                    in1=xbs("qsT", off=mc * 36, width=36),
                    op0=OP.add, op1=OP.mult)
            combine_chunk(2)

            transpose_chunk(1)
            pkS = []
            for mc in range(2):
                p = psum.tile([128, 288], F32, space="PSUM", tag="psA", bufs=3,
                              name=f"pkP{mc}")
                for kc in range(2):
                    nc.tensor.matmul(
                        out=p[:], lhsT=wfs("wk2", off=(kc * 2 + mc) * 128,
                                           width=128),
                        rhs=midK[kc][:], start=(kc == 0), stop=(kc == 1))
                t = pool.tile([128, 288], BF16, name=f"pkS{mc}")
                nc.scalar.activation(out=t[:], in_=p[:], func=AF.Identity,
                                     bias=wgs("bk2", off=mc, width=1))
                pkS.append(t)
            transpose_chunk(2)
            if debug:
                nc.sync.dma_start(out=dbg["d_posk0"][:], in_=pkS[0][:])
            samT = []
            for fc in range(2):
                t = pool.tile([128, 288], BF16, name=f"samT{fc}")
                nc.vector.tensor_copy(out=t[:], in_=samTP[fc][:])
                samT.append(t)
            # exp-table prefetch on ACT (deps long satisfied; queue slot here)
            wt2 = pool.tile([1, 1], F32)
            nc.scalar.activation(out=wt2[:], in_=kse[1][0:1, 0:1], func=AF.Exp)

            # ---- 12. sim tmps for the pos part (ready before conv finishes)
            simP = psum.tile([8, 288], F32, space="PSUM", tag="psA", bufs=3,
                             name="simP")
            tmps = []

            def sim_tmp(kap, qt, mc, i):
                tmp = pool.tile([128, 288], BF16, name=f"tmp{i}")
                qap = qt[:, mc * 36:(mc + 1) * 36]
                ta = tmp[:]
                nc.vector.tensor_tensor(
                    out=view3(ta, [[36, 8], [1, 36]]),
                    in0=AP(kap.tensor, kap.offset, [kap.ap[0], [36, 8], [1, 36]]),
                    in1=AP(qap.tensor, qap.offset, [qap.ap[0], [0, 8], [1, 36]]),
                    op=OP.mult)
                return tmp

            tmp_pos = [sim_tmp(pkS[0][:], pqS, 0, 0), sim_tmp(pkS[1][:], pqS, 1, 1)]
            nc.tensor.matmul(out=simP[:], lhsT=wfs("s0", width=8),
                             rhs=tmp_pos[0][:], start=True, stop=False,
                             skip_group_check=True)
            nc.tensor.matmul(out=simP[:], lhsT=wfs("s1", width=8),
                             rhs=tmp_pos[1][:], start=False, stop=False,
                             skip_group_check=True)

            # ---- 11b. conv (bf16), v-features first so the vS copies land
            # early; sim con-part matmuls interleave after convP0/convP1.
            convP = []
            for mc in range(4):
                p = psum.tile([128, 288], F32, space="PSUM", tag="convP",
                              bufs=4, name=f"convP{mc}")
                for kc in range(2):
                    nc.tensor.matmul(
                        out=p[:], lhsT=wfs("wcat", off=(kc * 4 + mc) * 128,
                                           width=128),
                        rhs=samT[kc][:], start=(kc == 0), stop=(kc == 1))
                convP.append(p)
            tmp_con = [sim_tmp(convP[0][:], cqS, 0, 2),
                       sim_tmp(convP[1][:], cqS, 1, 3)]
            nc.tensor.matmul(out=simP[:], lhsT=wfs("s0", width=8),
                             rhs=tmp_con[0][:], start=False, stop=False,
                             skip_group_check=True)
            nc.tensor.matmul(out=simP[:], lhsT=wfs("s1", width=8),
                             rhs=tmp_con[1][:], start=False, stop=True,
                             skip_group_check=True)
            vS = []
            for fc in range(2):
                t = pool.tile([128, 288], BF16, name=f"vS{fc}")
                nc.scalar.copy(out=t[:], in_=convP[2 + fc][:])
                vS.append(t)
            if debug:
                t = pool.tile([128, 288], F32)
                nc.scalar.copy(out=t[:], in_=convP[0][:])
                nc.sync.dma_start(out=dbg["d_conv0"][:], in_=t[:])
                t2 = pool.tile([8, 288], F32)
                nc.vector.tensor_copy(out=t2[:], in_=simP[:])
                nc.sync.dma_start(out=dbg["d_sim"][:], in_=t2[:])

            # ---- 13+14. softmax (deferred normalization: the head-expand of
            # the unnormalized exp runs on PE while sum/reciprocal run on DVE)
            ex = pool.tile([8, 288], BF16, name="ex")
            nc.scalar.activation(out=ex[:], in_=simP[:], func=AF.Exp)
            smt = pool.tile([8, 36], F32, name="smt")
            nc.vector.reduce_sum(out=smt[:], in_=view3(ex[:], [[1, 36], [36, 8]]),
                                 axis=mybir.AxisListType.X)
            rct = pool.tile([8, 36], BF16, name="rct")
            with nc.allow_low_precision(reason="bf16 softmax norm is well "
                                        "within the 2e-2 tolerance"):
                nc.vector.reciprocal(out=rct[:], in_=smt[:])
            if debug:
                exn = pool.tile([8, 288], BF16, name="exn")
                rca = rct[:]
                nc.vector.tensor_tensor(
                    out=view3(exn[:], [[1, 36], [36, 8]]),
                    in0=view3(ex[:], [[1, 36], [36, 8]]),
                    in1=AP(rca.tensor, rca.offset, [rca.ap[0], [1, 36], [0, 8]]),
                    op=OP.mult)
                nc.sync.dma_start(out=dbg["d_at"][:], in_=exn[:])

            aeP, reP = [], []
            for fc in range(2):
                ae = psum.tile([128, 288], F32, space="PSUM", tag="psA",
                               bufs=3, name=f"aeP{fc}")
                nc.tensor.matmul(out=ae[:], lhsT=wfs(f"e{fc}", rows=8,
                                                     width=128),
                                 rhs=ex[:], start=True, stop=True)
                aeP.append(ae)
            for fc in range(2):
                re = psum.tile([128, 288], F32, space="PSUM", tag="psA",
                               bufs=3, name=f"reP{fc}")
                nc.tensor.matmul(out=re[:, :36], lhsT=wfs(f"e{fc}", rows=8,
                                                          width=128),
                                 rhs=rct[:], start=True, stop=True)
                reP.append(re)
            avT = []
            for fc in range(2):
                pr = pool.tile([128, 288], BF16, name=f"pr{fc}")
                nc.vector.tensor_tensor(out=pr[:], in0=vS[fc][:],
                                        in1=aeP[fc][:], op=OP.mult)
                au = pool.tile([128, 36], BF16, name=f"avu{fc}")
                with nc.allow_low_precision(reason="bf16 attn output is well "
                                            "within the 2e-2 tolerance"):
                    nc.vector.reduce_sum(out=au[:],
                                         in_=view3(pr[:], [[1, 36], [36, 8]]),
                                         axis=mybir.AxisListType.X)
                av = pool.tile([128, 36], BF16, name=f"avT{fc}")
                nc.vector.tensor_tensor(out=av[:], in0=au[:],
                                        in1=reP[fc][:, :36], op=OP.mult)
                avT.append(av)
            if debug:
                nc.sync.dma_start(out=dbg["d_av0"][:], in_=avT[0][:])

            # ---- 15. out = attn_out @ W_out + b_out + identity, one fused
            # DVE op per half then straight to DMA.
            oP = psum.tile([128, 288], F32, space="PSUM", tag="psA", bufs=3,
                           name="oP")
            oT = pool.tile([128, 72], F32, name="oT")
            oda = out[:]
            for mc in range(2):
                for kc in range(2):
                    nc.tensor.matmul(
                        out=oP[:, mc * 36:(mc + 1) * 36],
                        lhsT=wfs("wout", off=(kc * 2 + mc) * 128, width=128),
                        rhs=avT[kc][:], start=(kc == 0), stop=(kc == 1))
                nc.vector.scalar_tensor_tensor(
                    out=oT[:, mc * 36:(mc + 1) * 36],
                    in0=oP[:, mc * 36:(mc + 1) * 36],
                    scalar=wgs("bout", off=mc, width=1),
                    in1=xbs("deT", off=mc * 36, width=36),
                    op0=OP.add, op1=OP.add)
                ota = oT[:, mc * 36:(mc + 1) * 36]
                nc.sync.dma_start(
                    out=AP(oda.tensor, oda.offset + mc * 128 * 36,
                           [[36, 128], [1, 36]]),
                    in_=AP(ota.tensor, ota.offset, [[72, 128], [1, 36]]))

    return nc


# ------------------------------------------------------------------- driver

def make_in_maps(dec_embed, bev_feat, query_scale, ref_points, weights):
    wf = pack_wf(weights)
    wg = pack_wg(weights)
    bevs = []
    for b in range(B):
        hwc = bev_feat[b].transpose(1, 2, 0).reshape(H * W, 256)
        bev_hwc = np.zeros((H * W, 512), np.float32)
        bev_hwc[:, 0:256] = hwc
        bev_hwc[:(H - 1) * W, 256:512] = hwc[W:]
        bevs.append(np.ascontiguousarray(bev_hwc.astype(NPBF)))
    in_maps = []
    for c in range(8):
        b, kh = c // 2, c % 2
        in_maps.append({
            "bev": bevs[b], "wf": wf, "wg": wg,
            "xbl": pack_xb(dec_embed, query_scale, ref_points, b, 3 * kh),
            "xh": pack_xh(dec_embed, b, 3 * kh),
        })
    return in_maps


def assemble_output(results):
    out = np.zeros((K, B, T, DIM), np.float32)
    for c in range(8):
        b, kh = c // 2, c % 2
        oc = results[c]["out"]                     # (256, 36)
        out[3 * kh:3 * kh + 3, b] = oc.T.reshape(3, T, DIM)
    return out


_WNAMES = ["W_con_q", "b_con_q", "W_con_k", "W_v", "Wq1", "bq1", "Wq2", "bq2",
           "Wk1", "bk1", "Wk2", "bk2", "Wo1", "bo1", "Wo2", "bo2",
           "W_out", "b_out"]


def kernel(**inputs):
    from concourse.bass_utils import run_bass_kernel_spmd
    dec_embed = np.asarray(inputs["dec_embed"], np.float32)
    bev_feat = np.asarray(inputs["bev_feat"], np.float32)
    query_scale = np.asarray(inputs["query_scale"], np.float32)
    ref_points = np.asarray(inputs["ref_points"], np.float32)
    weights = {n: np.asarray(inputs[n], np.float32) for n in _WNAMES}

    nc = build_nc(sim_mode=False, debug=False)
    split_multiwaits(nc)
    in_maps = make_in_maps(dec_embed, bev_feat, query_scale, ref_points, weights)
    res = run_bass_kernel_spmd(nc, in_maps, list(range(8)))
    return assemble_output(res.results)
